# revision 6
# baseline (speedup 1.0000x reference)
"""Distributed 3-layer GCN (AqSolModel) on 8 TRN2 NeuronCores.

Strategy
--------
Nodes are partitioned by graph id (2048 graphs -> 256 graphs/core, nodes of a
graph never cross cores, so the segment-mean pool is core-local).  Per layer:

  z = (h @ W) scaled per-row by G_l*dis (dis=1/sqrt(deg); G_l is a per-layer
  gain that keeps fp8 values in normal range), stored as one fp8 row-table
  per half-window (A = tiles [0,T_half), B = rest) so table row ids fit
  int16; AllGather both tables across the 8 cores; per GROUP of 4 dst tiles,
  two dma_gather calls (one per source table, ~4.4k rows each, cycled over
  the 4 SWDGE queues so descriptor generation overlaps across Q7 core
  pairs) fetch the group's in-edge source rows; host-precomputed one-hot
  sel matrices stream from DRAM and PE segment-sums the gathered rows per
  dst tile; the self-loop term is an identity matmul from the SBUF-resident
  z store, and h = relu(dis/G_l * agg + b) is one ACT op (bias folded in as
  a K=1 outer-product matmul with the sqrt(deg) row, so GCN's symmetric
  norm comes out exactly).  The segment-mean pool + MLP head run per-core
  in a transposed layout (graphs never cross cores).
"""

import sys
import numpy as np

sys.path.insert(0, "/opt/trn_rl_repo")

import ml_dtypes

import concourse.bass as bass
import concourse.bacc as bacc
import concourse.mybir as mybir
import concourse.tile as tile
from concourse.masks import make_identity

N_NODES = 50000
N_EDGES = 800000
N_GRAPHS = 2048
N_FEAT = 64
HIDDEN = 256
N_CORES = 8
GPC = N_GRAPHS // N_CORES          # graphs per core (256)
GPW = GPC // 2                     # graphs per window (128)
GS = 4                             # dst tiles per gather group

F32 = mybir.dt.float32
BF16 = mybir.dt.bfloat16
FP8 = mybir.dt.float8e4
GDT = FP8            # gather-table dtype (z table, gathered rows, sel)
GAINS = (64.0, 1024.0, 8192.0)   # per-layer fp8 dynamic-range gains
I32 = mybir.dt.int32
I16 = mybir.dt.int16
BF16_NP = ml_dtypes.bfloat16
FP8_NP = ml_dtypes.float8_e4m3


# ---------------------------------------------------------------- host side --

def preprocess(x, edge_index, batch):
    """Shard the graph across cores.  Returns (cfg, per-core input arrays)."""
    x = np.asarray(x, np.float32)
    src_g = np.asarray(edge_index[0], np.int64)
    dst_g = np.asarray(edge_index[1], np.int64)
    batch = np.asarray(batch, np.int64)

    # node -> core / half-window, contiguous because batch is sorted
    gsplit = np.searchsorted(batch, np.arange(0, N_GRAPHS + 1, GPW))  # 17 cuts
    half_cnt = np.diff(gsplit)                       # nodes per (core, half)
    T_half = int(np.max((half_cnt + 127) // 128))
    T_tiles = 2 * T_half
    n_c = T_tiles * 128                              # node slots per core
    nh = T_half * 128                                # node slots per window
    assert N_CORES * nh < 2 ** 15                    # table row ids fit int16

    # slot of each node inside its core
    core_of = np.repeat(np.arange(16) // 2, half_cnt)           # per node
    half_of = np.repeat(np.arange(16) % 2, half_cnt)
    rank_in_half = np.arange(N_NODES) - np.repeat(gsplit[:-1], half_cnt)
    slot = half_of * nh + rank_in_half

    # degree (in-degree + self loop) and norm factors
    deg = np.bincount(dst_g, minlength=N_NODES).astype(np.float64) + 1.0
    dis = (1.0 / np.sqrt(deg)).astype(np.float32)
    invdis = np.sqrt(deg).astype(np.float32)

    # real edges only; the self-loop term (z[v] into agg[v]) is applied on
    # device as an identity matmul from the SBUF-resident z store
    e_core = core_of[dst_g]
    e_tile = slot[dst_g] // 128
    e_local = slot[dst_g] % 128
    # src window: table A rows are window-0 slots, table B window-1
    e_win = half_of[src_g]                                       # 0=A, 1=B
    e_row = (core_of[src_g] * nh + (slot[src_g] % nh)).astype(np.int64)

    # sort edges by (dst core, dst tile, src window, src row)
    order = np.lexsort((e_row, e_win, e_tile, e_core))
    e_core, e_tile, e_local, e_win, e_row = (
        a[order] for a in (e_core, e_tile, e_local, e_win, e_row))

    counts = np.zeros((N_CORES, T_tiles, 2), np.int64)
    np.add.at(counts, (e_core, e_tile, e_win), 1)
    gw = ((counts.max(axis=0) + 127) // 128).astype(np.int64)    # [T,2] ktiles

    # global K-tile enumeration: per group of GS tiles, first all A-window
    # segments (one gather call), then all B-window segments (second call).
    n_groups = (T_tiles + GS - 1) // GS
    seg_off = np.zeros((T_tiles, 2), np.int64)   # K-tile col of each (t,w)
    call_cols = []                               # (start, ncols) per call
    kcur = 0
    for g in range(n_groups):
        ts = range(g * GS, min((g + 1) * GS, T_tiles))
        for w in range(2):
            c0 = kcur
            for t in ts:
                seg_off[t, w] = kcur
                kcur += int(gw[t, w])
            call_cols.append((c0, kcur - c0))
    t_kt = kcur

    # pack wrapped int16 gather indices [128, t_kt*8] and host-built one-hot
    # sel tables [128, t_kt*128] (value 1 at [stream slot, ktile*128+dst])
    gidx = np.zeros((N_CORES, 16, t_kt * 8), np.int16)
    selh = np.zeros((N_CORES, 128, t_kt * 128), np.uint8)
    bucket = (e_core * T_tiles + e_tile) * 2 + e_win
    bb = np.zeros(N_CORES * T_tiles * 2 + 1, np.int64)
    np.add.at(bb, bucket + 1, 1)
    bb = np.cumsum(bb)
    pos = np.arange(len(e_row)) - bb[bucket]     # rank within (t, w) bucket
    # stream position of this edge: seg base + rank
    spos = seg_off[e_tile, e_win] * 128 + pos
    gidx[e_core, spos % 16, spos // 16] = e_row.astype(np.int16)
    selh[e_core, spos % 128, (spos // 128) * 128 + e_local] = 1
    selh = selh.astype(FP8_NP)

    # per-core node-side arrays
    xT = np.zeros((N_CORES, N_FEAT, n_c), BF16_NP)
    dis_a = np.zeros((N_CORES, 128, T_tiles), np.float32)
    invdis_a = np.zeros((N_CORES, 1, n_c), BF16_NP)
    batchloc = np.full((N_CORES, 128, T_tiles), -1, np.float32)
    xT[core_of, :, slot] = x.astype(BF16_NP)        # fancy: (N, F) -> rows
    dis_a[core_of, slot % 128, slot // 128] = dis
    invdis_a[core_of, 0, slot] = invdis.astype(BF16_NP)
    batchloc[core_of, slot % 128, slot // 128] = (batch % GPW).astype(np.float32)

    # per-graph 1/max(count,1), [1, 256] per core (window-major)
    gcnt = np.bincount(batch, minlength=N_GRAPHS).astype(np.float32)
    rec = (1.0 / np.maximum(gcnt, 1.0)).reshape(N_CORES, 1, GPC)
    rec = np.ascontiguousarray(np.broadcast_to(rec, (N_CORES, 128, GPC)))

    # per-layer fp8 gain: z stored as G_l * z, undone in the epilogue
    disg = np.concatenate([dis_a * g for g in GAINS], axis=2)     # z-store
    disu = np.concatenate([dis_a / g for g in GAINS], axis=2)     # epilogue

    cfg = dict(T_half=T_half,
               gw=tuple((int(a), int(b)) for a, b in gw),
               calls=tuple((int(a), int(b)) for a, b in call_cols),
               seg=tuple((int(a), int(b)) for a, b in seg_off))
    gidx = np.tile(gidx, (1, 8, 1))
    arrays = dict(xT=xT, disg=disg, disu=disu, invdis=invdis_a,
                  batchloc=batchloc, gidx=gidx, selh=selh, rec=rec)
    return cfg, arrays


def pack_weights(W1, b1, W2, b2, W3, b3, Wl, bl, Wo, bo):
    """Pack the (replicated) weights into device layouts."""
    def to16(a):
        return np.asarray(a, np.float32).astype(BF16_NP)

    w1 = np.zeros((128, HIDDEN), BF16_NP)
    w1[:N_FEAT] = to16(W1)                                     # [64,256] pad K
    w2 = np.asarray(W2, np.float32).reshape(2, 128, HIDDEN)
    w2 = np.concatenate([to16(w2[0]), to16(w2[1])], axis=1)    # [128, 512]
    w3 = np.asarray(W3, np.float32).reshape(2, 128, HIDDEN)
    w3 = np.concatenate([to16(w3[0]), to16(w3[1])], axis=1)
    wl = np.asarray(Wl, np.float32).reshape(2, 128, 128)
    wl = np.concatenate([to16(wl[0]), to16(wl[1])], axis=1)    # [128, 256]
    wo = to16(np.asarray(Wo, np.float32).reshape(128, 1))      # [128, 1]
    b123 = np.stack([np.asarray(b, np.float32) * g
                     for b, g in zip((b1, b2, b3), GAINS)])
    b123 = b123.astype(BF16_NP).reshape(1, 3 * HIDDEN)         # [1, 768]
    bl_a = np.asarray(bl, np.float32).reshape(128, 1)          # [128, 1]
    bo_a = np.asarray(bo, np.float32).reshape(1, 1)
    return dict(w1=w1, w2=w2, w3=w3, wl=wl, wo=wo, b123=b123,
                bl=bl_a, bo=bo_a)


# -------------------------------------------------------------- bass kernel --

def build(cfg):
    """Build the SPMD Bass program (same graph on all 8 cores)."""
    T_half = cfg["T_half"]
    gw = cfg["gw"]
    calls = cfg["calls"]
    seg = cfg["seg"]
    T_tiles = 2 * T_half
    n_c = T_tiles * 128
    nh = T_half * 128
    t_kt = int(sum(a + b for a, b in gw))
    n_groups = (T_tiles + GS - 1) // GS

    nc = bacc.Bacc(None, target_bir_lowering=False,
                   dynamic_dma_scratch_size=49152,
                   num_swdge_queues=4)
    P = {}
    P["xT"] = nc.declare_dram_parameter("xT", [N_FEAT, n_c], BF16, False)
    P["disg"] = nc.declare_dram_parameter("disg", [128, 3 * T_tiles], F32, False)
    P["disu"] = nc.declare_dram_parameter("disu", [128, 3 * T_tiles], F32, False)
    P["invdis"] = nc.declare_dram_parameter("invdis", [1, n_c], BF16, False)
    P["batchloc"] = nc.declare_dram_parameter("batchloc", [128, T_tiles], F32, False)
    P["gidx"] = nc.declare_dram_parameter("gidx", [128, t_kt * 8], I16, False)
    selh_d = nc.declare_dram_parameter("selh", [128, t_kt * 128], FP8, False)
    P["rec"] = nc.declare_dram_parameter("rec", [128, GPC], F32, False)
    P["w1"] = nc.declare_dram_parameter("w1", [128, HIDDEN], BF16, False)
    P["w2"] = nc.declare_dram_parameter("w2", [128, 2 * HIDDEN], BF16, False)
    P["w3"] = nc.declare_dram_parameter("w3", [128, 2 * HIDDEN], BF16, False)
    P["wl"] = nc.declare_dram_parameter("wl", [128, 256], BF16, False)
    P["wo"] = nc.declare_dram_parameter("wo", [128, 1], BF16, False)
    P["b123"] = nc.declare_dram_parameter("b123", [1, 3 * HIDDEN], BF16, False)
    P["bl"] = nc.declare_dram_parameter("bl", [128, 1], F32, False)
    P["bo"] = nc.declare_dram_parameter("bo", [1, 1], F32, False)
    out_ext = nc.declare_dram_parameter("out", [1, GPC], F32, True)

    rg = [list(range(N_CORES))]
    AF = mybir.ActivationFunctionType
    OP = mybir.AluOpType

    with tile.TileContext(nc) as tc:
        with (
            tc.tile_pool(name="const", bufs=1) as cpool,
            tc.tile_pool(name="big", bufs=1) as bigpool,
            tc.tile_pool(name="work", bufs=3) as wpool,
            tc.tile_pool(name="gath", bufs=2) as gpool,
            tc.tile_pool(name="sel", bufs=2) as spool,
            tc.tile_pool(name="ps", bufs=2, space="PSUM") as pspool,
            tc.tile_pool(name="psa", bufs=2, space="PSUM") as papool,
            tc.tile_pool(name="pspool", bufs=1, space="PSUM") as ppool,
            tc.tile_pool(name="dram", bufs=2, space="DRAM") as dpool,
        ):
            # ---- constants / parameter loads (once) ----
            iota_t = cpool.tile([128, 128], F32)
            nc.gpsimd.iota(iota_t[:], pattern=[[1, 128]], base=0,
                           channel_multiplier=0,
                           allow_small_or_imprecise_dtypes=True)
            ident = cpool.tile([128, 128], BF16)
            make_identity(nc, ident[:])
            ident8 = cpool.tile([128, 128], GDT)
            nc.vector.tensor_copy(ident8[:], ident[:])

            def load(name, shape, dt):
                t = cpool.tile(list(shape), dt, name=f"sb_{name}")
                nc.sync.dma_start(out=t[:], in_=P[name][:, :])
                return t

            xT_sb = bigpool.tile([N_FEAT, n_c], BF16)
            nc.sync.dma_start(out=xT_sb[:], in_=P["xT"][:, :])
            disg_sb = load("disg", (128, 3 * T_tiles), F32)
            disu_sb = load("disu", (128, 3 * T_tiles), F32)
            invdis_sb = load("invdis", (1, n_c), BF16)
            batchloc_sb = load("batchloc", (128, T_tiles), F32)
            gidx_sb = bigpool.tile([128, t_kt * 8], I16)
            nc.sync.dma_start(out=gidx_sb[:], in_=P["gidx"][:, :])
            rec_sb = load("rec", (128, GPC), F32)
            w1_sb = load("w1", (128, HIDDEN), BF16)
            w2_sb = load("w2", (128, 2 * HIDDEN), BF16)
            w3_sb = load("w3", (128, 2 * HIDDEN), BF16)
            wl_sb = load("wl", (128, 256), BF16)
            wo_sb = load("wo", (128, 1), BF16)
            b123_sb = load("b123", (1, 3 * HIDDEN), BF16)
            bl_sb = load("bl", (128, 1), F32)
            bo_sb = load("bo", (1, 1), F32)

            # persistent transposed activations for the dense matmuls, and
            # the SBUF-resident z store (self-loop source)
            hT0 = bigpool.tile([128, n_c], BF16)
            hT1 = bigpool.tile([128, n_c], BF16)
            zstore = bigpool.tile([128, T_tiles * HIDDEN], GDT)

            pool_ps = [None, None]      # [chunk] psum tiles for poolT, per win
            out_sb = cpool.tile([1, GPC], F32)
            qno = [0]

            for layer in range(3):
                w_sb = (w1_sb, w2_sb, w3_sb)[layer]
                # -------- dense: z = dis * (h @ W), fp8, to DRAM tables -----
                zloc = [dpool.tile([nh, HIDDEN], GDT, tag=f"zloc{w}",
                                   name=f"zloc{w}") for w in range(2)]
                for t in range(T_tiles):
                    sl = slice(t * 128, (t + 1) * 128)
                    psz = pspool.tile([128, HIDDEN], F32, tag="mm")
                    if layer == 0:
                        nc.tensor.matmul(psz[:], lhsT=xT_sb[:, sl],
                                         rhs=w_sb[:N_FEAT, :HIDDEN],
                                         start=True, stop=True)
                    else:
                        nc.tensor.matmul(psz[:], lhsT=hT0[:, sl],
                                         rhs=w_sb[:, 0:HIDDEN],
                                         start=True, stop=False)
                        nc.tensor.matmul(psz[:], lhsT=hT1[:, sl],
                                         rhs=w_sb[:, HIDDEN:2 * HIDDEN],
                                         start=False, stop=True)
                    zsl = zstore[:, t * HIDDEN:(t + 1) * HIDDEN]
                    nc.scalar.activation(
                        zsl, psz[:], AF.Copy,
                        scale=disg_sb[:, layer * T_tiles + t:
                                      layer * T_tiles + t + 1])
                    hsl = slice((t % T_half) * 128, (t % T_half + 1) * 128)
                    nc.sync.dma_start(out=zloc[t // T_half][hsl, :], in_=zsl)

                # -------- AllGather the scaled z tables ---------------------
                zfull = [dpool.tile([N_CORES * nh, HIDDEN], GDT,
                                    tag=f"zfull{w}", name=f"zfull{w}",
                                    addr_space="Shared")
                         for w in range(2)]
                for w in range(2):
                    nc.gpsimd.collective_compute(
                        "AllGather", OP.bypass, replica_groups=rg,
                        ins=[zloc[w].opt()], outs=[zfull[w].opt()])

                # -------- per group: 2 gathers + sel load + per-tile PE -----
                b_row = b123_sb[0:1, layer * HIDDEN:(layer + 1) * HIDDEN]
                for g in range(n_groups):
                    ts = range(g * GS, min((g + 1) * GS, T_tiles))
                    ca, cb = calls[2 * g], calls[2 * g + 1]
                    gath = [None, None]
                    for w, (c0, nk) in enumerate((ca, cb)):
                        if nk == 0:
                            continue
                        gt = gpool.tile([128, nk * HIDDEN], GDT,
                                        tag=f"gath{w}")
                        gv = gt[:].rearrange("p (k h) -> p k h", h=HIDDEN)
                        nidx = nk * 128
                        nc.gpsimd.dma_gather(
                            out_ap=gv, in_ap=zfull[w][:, :],
                            idxs_ap=gidx_sb[:, c0 * 8:(c0 + nk) * 8],
                            num_idxs=nidx, num_idxs_reg=nidx,
                            elem_size=HIDDEN, single_packet=False,
                            queue_num=qno[0] % 4)
                        qno[0] += 1
                        gath[w] = (gt, c0)
                    c0 = ca[0]
                    nksum = ca[1] + cb[1]
                    sel_sb = spool.tile([128, nksum * 128], GDT, tag="sel")
                    nc.sync.dma_start(
                        out=sel_sb[:],
                        in_=selh_d[:, c0 * 128:(c0 + nksum) * 128])

                    for t in ts:
                        sl = slice(t * 128, (t + 1) * 128)
                        psa = papool.tile([128, HIDDEN], F32, tag="psa")
                        # bias as outer product (1/dis)[dst] x b -> exact norm
                        nc.tensor.matmul(
                            psa[:], lhsT=invdis_sb[0:1, sl], rhs=b_row,
                            start=True, stop=False)
                        # self-loop term: agg[v] += z[v], from the z store
                        nc.tensor.matmul(
                            psa[:], lhsT=ident8[:],
                            rhs=zstore[:, t * HIDDEN:(t + 1) * HIDDEN],
                            start=False,
                            stop=(gw[t][0] + gw[t][1] == 0))
                        for w in range(2):
                            gwt = gw[t][w]
                            if gwt == 0:
                                continue
                            gt, gc0 = gath[w]
                            off = seg[t][w] - gc0
                            last = (w == 1) or gw[t][1] == 0
                            for j in range(gwt):
                                nc.tensor.matmul(
                                    psa[:],
                                    lhsT=sel_sb[:, (seg[t][w] - c0 + j) * 128:
                                                (seg[t][w] - c0 + j + 1) * 128],
                                    rhs=gt[:, (off + j) * HIDDEN:
                                           (off + j + 1) * HIDDEN],
                                    start=False,
                                    stop=(last and j == gwt - 1))
                        # h = relu(dis * agg + b)
                        h_sb = wpool.tile([128, HIDDEN], BF16, tag="h")
                        nc.scalar.activation(
                            h_sb[:], psa[:], AF.Relu,
                            scale=disu_sb[:, layer * T_tiles + t:
                                          layer * T_tiles + t + 1])
                        if layer < 2:
                            for cch in range(2):
                                pst = pspool.tile([128, 128], BF16, tag="mm")
                                nc.tensor.transpose(
                                    pst[:], h_sb[:, cch * 128:(cch + 1) * 128],
                                    ident[:])
                                hT = (hT0, hT1)[cch]
                                nc.vector.tensor_copy(hT[:, sl], pst[:])
                        else:
                            win = t // T_half
                            first = (t % T_half) == 0
                            last = (t % T_half) == T_half - 1
                            if first:
                                pool_ps[0] = ppool.tile([128, 128], F32,
                                                        name="poolT0",
                                                        tag="poolT0", bufs=1)
                                pool_ps[1] = ppool.tile([128, 128], F32,
                                                        name="poolT1",
                                                        tag="poolT1", bufs=1)
                            selp = spool.tile([128, 128], BF16, tag="selp")
                            nc.vector.tensor_tensor(
                                selp[:],
                                batchloc_sb[:, t:t + 1].to_broadcast([128, 128]),
                                iota_t[:], op=OP.is_equal)
                            for cch in range(2):
                                nc.tensor.matmul(
                                    pool_ps[cch][:],
                                    lhsT=h_sb[:, cch * 128:(cch + 1) * 128],
                                    rhs=selp[:], start=first, stop=last)
                            if last:
                                # ---- head for this window of 128 graphs ----
                                rrow = rec_sb[:, win * GPW:(win + 1) * GPW]
                                psu = papool.tile([128, GPW], F32, tag="head",
                                                  bufs=1)
                                for cch in range(2):
                                    gT = wpool.tile([128, GPW], BF16,
                                                    tag="gT")
                                    nc.vector.tensor_tensor(
                                        gT[:], pool_ps[cch][:, :GPW],
                                        rrow, op=OP.mult)
                                    nc.tensor.matmul(
                                        psu[:],
                                        lhsT=wl_sb[:, cch * 128:(cch + 1) * 128],
                                        rhs=gT[:], start=(cch == 0),
                                        stop=(cch == 1))
                                uT = wpool.tile([128, GPW], BF16, tag="uT")
                                nc.scalar.activation(uT[:], psu[:], AF.Relu,
                                                     bias=bl_sb[:, 0:1])
                                pso = papool.tile([1, GPW], F32, tag="head",
                                                  bufs=1)
                                nc.tensor.matmul(pso[:], lhsT=wo_sb[:, 0:1],
                                                 rhs=uT[:], start=True,
                                                 stop=True)
                                nc.vector.tensor_scalar(
                                    out_sb[0:1, win * GPW:(win + 1) * GPW],
                                    pso[:], bo_sb[0:1, 0:1], None, op0=OP.add)
            nc.sync.dma_start(out=out_ext[:, :], in_=out_sb[:])
    nc.finalize()
    return nc


# ------------------------------------------------------------------ runner --

_CACHE = {}


def _get_program(cfg):
    key = (cfg["T_half"], cfg["gw"])
    if key not in _CACHE:
        _CACHE[key] = build(cfg)
    return _CACHE[key]


def kernel(x, edge_index, batch, W1, b1, W2, b2, W3, b3, Wl, bl, Wo, bo):
    from concourse.bass_utils import run_bass_kernel_spmd

    cfg, arrays = preprocess(x, edge_index, batch)
    wts = pack_weights(W1, b1, W2, b2, W3, b3, Wl, bl, Wo, bo)
    nc = _get_program(cfg)

    in_maps = []
    for c in range(N_CORES):
        m = {k: np.ascontiguousarray(v[c]) for k, v in arrays.items()}
        m.update(wts)
        in_maps.append(m)

    res = run_bass_kernel_spmd(nc, in_maps, core_ids=list(range(N_CORES)))
    outs = res.results
    out = np.concatenate([outs[c]["out"].reshape(GPC) for c in range(N_CORES)])
    return out.reshape(N_GRAPHS, 1).astype(np.float32)


# revision 10
# speedup vs baseline: 1.0693x; 1.0693x over previous
"""Distributed 3-layer GCN (AqSolModel) on 8 TRN2 NeuronCores.

Strategy
--------
Nodes are partitioned by graph id (2048 graphs -> 256 graphs/core, nodes of a
graph never cross cores, so the segment-mean pool is core-local).  Per layer:

  z = (h @ W) scaled per-row by G_l*dis (dis=1/sqrt(deg); G_l is a per-layer
  gain that keeps fp8 values in normal range), stored as one fp8 row-table
  per half-window (A = tiles [0,T_half), B = rest) so table row ids fit
  int16; AllGather both tables across the 8 cores; per GROUP of 4 dst tiles,
  two dma_gather calls (one per source table, ~4.4k rows each, cycled over
  the 4 SWDGE queues so descriptor generation overlaps across Q7 core
  pairs) fetch the group's in-edge source rows; host-precomputed one-hot
  sel matrices stream from DRAM and PE segment-sums the gathered rows per
  dst tile; the self-loop term is an identity matmul from the SBUF-resident
  z store, and h = relu(dis/G_l * agg + b) is one ACT op (bias folded in as
  a K=1 outer-product matmul with the sqrt(deg) row, so GCN's symmetric
  norm comes out exactly).  The segment-mean pool + MLP head run per-core
  in a transposed layout (graphs never cross cores).
"""

import sys
import numpy as np

sys.path.insert(0, "/opt/trn_rl_repo")

import ml_dtypes

import concourse.bass as bass
import concourse.bacc as bacc
import concourse.mybir as mybir
import concourse.tile as tile
from concourse.masks import make_identity

N_NODES = 50000
N_EDGES = 800000
N_GRAPHS = 2048
N_FEAT = 64
HIDDEN = 256
N_CORES = 8
GPC = N_GRAPHS // N_CORES          # graphs per core (256)
GPW = GPC // 2                     # graphs per window (128)
GS = 4                             # dst tiles per gather group

F32 = mybir.dt.float32
BF16 = mybir.dt.bfloat16
FP8 = mybir.dt.float8e4
GDT = FP8            # gather-table dtype (z table, gathered rows, sel)
GAINS = (64.0, 1024.0, 8192.0)   # per-layer fp8 dynamic-range gains
I32 = mybir.dt.int32
I16 = mybir.dt.int16
BF16_NP = ml_dtypes.bfloat16
FP8_NP = ml_dtypes.float8_e4m3


# ---------------------------------------------------------------- host side --

def preprocess(x, edge_index, batch):
    """Shard the graph across cores.  Returns (cfg, per-core input arrays)."""
    x = np.asarray(x, np.float32)
    src_g = np.asarray(edge_index[0], np.int64)
    dst_g = np.asarray(edge_index[1], np.int64)
    batch = np.asarray(batch, np.int64)

    # node -> core / half-window, contiguous because batch is sorted
    gsplit = np.searchsorted(batch, np.arange(0, N_GRAPHS + 1, GPW))  # 17 cuts
    half_cnt = np.diff(gsplit)                       # nodes per (core, half)
    T_half = int(np.max((half_cnt + 127) // 128))
    T_tiles = 2 * T_half
    n_c = T_tiles * 128                              # node slots per core
    nh = T_half * 128                                # node slots per window
    assert N_CORES * nh < 2 ** 15                    # table row ids fit int16

    # slot of each node inside its core
    core_of = np.repeat(np.arange(16) // 2, half_cnt)           # per node
    half_of = np.repeat(np.arange(16) % 2, half_cnt)
    rank_in_half = np.arange(N_NODES) - np.repeat(gsplit[:-1], half_cnt)
    slot = half_of * nh + rank_in_half

    # degree (in-degree + self loop) and norm factors
    deg = np.bincount(dst_g, minlength=N_NODES).astype(np.float64) + 1.0
    dis = (1.0 / np.sqrt(deg)).astype(np.float32)
    invdis = np.sqrt(deg).astype(np.float32)

    # real edges only; the self-loop term (z[v] into agg[v]) is applied on
    # device as an identity matmul from the SBUF-resident z store
    e_core = core_of[dst_g]
    e_tile = slot[dst_g] // 128
    e_local = slot[dst_g] % 128
    # src window: table A rows are window-0 slots, table B window-1
    e_win = half_of[src_g]                                       # 0=A, 1=B
    e_row = (core_of[src_g] * nh + (slot[src_g] % nh)).astype(np.int64)

    # sort edges by (dst core, dst tile, src window, src row)
    order = np.lexsort((e_row, e_win, e_tile, e_core))
    e_core, e_tile, e_local, e_win, e_row = (
        a[order] for a in (e_core, e_tile, e_local, e_win, e_row))

    counts = np.zeros((N_CORES, T_tiles, 2), np.int64)
    np.add.at(counts, (e_core, e_tile, e_win), 1)
    gw = ((counts.max(axis=0) + 127) // 128).astype(np.int64)    # [T,2] ktiles

    # global K-tile enumeration: per group of GS tiles, first all A-window
    # segments (one gather call), then all B-window segments (second call).
    n_groups = (T_tiles + GS - 1) // GS
    seg_off = np.zeros((T_tiles, 2), np.int64)   # K-tile col of each (t,w)
    call_cols = []                               # (start, ncols) per call
    kcur = 0
    for g in range(n_groups):
        ts = range(g * GS, min((g + 1) * GS, T_tiles))
        for w in range(2):
            c0 = kcur
            for t in ts:
                seg_off[t, w] = kcur
                kcur += int(gw[t, w])
            call_cols.append((c0, kcur - c0))
    t_kt = kcur

    # pack wrapped int16 gather indices [128, t_kt*8] and host-built one-hot
    # sel tables [128, t_kt*128] (value 1 at [stream slot, ktile*128+dst])
    gidx = np.zeros((N_CORES, 16, t_kt * 8), np.int16)
    selh = np.zeros((N_CORES, 128, t_kt * 128), np.uint8)
    bucket = (e_core * T_tiles + e_tile) * 2 + e_win
    bb = np.zeros(N_CORES * T_tiles * 2 + 1, np.int64)
    np.add.at(bb, bucket + 1, 1)
    bb = np.cumsum(bb)
    pos = np.arange(len(e_row)) - bb[bucket]     # rank within (t, w) bucket
    # stream position of this edge: seg base + rank
    spos = seg_off[e_tile, e_win] * 128 + pos
    gidx[e_core, spos % 16, spos // 16] = e_row.astype(np.int16)
    selh[e_core, spos % 128, (spos // 128) * 128 + e_local] = 1
    selh = selh.astype(FP8_NP)

    # per-core node-side arrays
    xT = np.zeros((N_CORES, N_FEAT, n_c), BF16_NP)
    dis_a = np.zeros((N_CORES, 128, T_tiles), np.float32)
    invdis_a = np.zeros((N_CORES, 1, n_c), BF16_NP)
    batchloc = np.full((N_CORES, 128, T_tiles), -1, np.float32)
    xT[core_of, :, slot] = x.astype(BF16_NP)        # fancy: (N, F) -> rows
    dis_a[core_of, slot % 128, slot // 128] = dis
    invdis_a[core_of, 0, slot] = invdis.astype(BF16_NP)
    batchloc[core_of, slot % 128, slot // 128] = (batch % GPW).astype(np.float32)

    # per-graph 1/max(count,1), [1, 256] per core (window-major)
    gcnt = np.bincount(batch, minlength=N_GRAPHS).astype(np.float32)
    rec = (1.0 / np.maximum(gcnt, 1.0)).reshape(N_CORES, 1, GPC)
    rec = np.ascontiguousarray(np.broadcast_to(rec, (N_CORES, 128, GPC)))

    # per-layer fp8 gain: z stored as G_l * z, undone in the epilogue
    disg = np.concatenate([dis_a * g for g in GAINS], axis=2)     # z-store
    disu = np.concatenate([dis_a / g for g in GAINS], axis=2)     # epilogue

    cfg = dict(T_half=T_half,
               gw=tuple((int(a), int(b)) for a, b in gw),
               calls=tuple((int(a), int(b)) for a, b in call_cols),
               seg=tuple((int(a), int(b)) for a, b in seg_off))
    gidx = np.tile(gidx, (1, 8, 1))
    arrays = dict(xT=xT, disg=disg, disu=disu, invdis=invdis_a,
                  batchloc=batchloc, gidx=gidx, selh=selh, rec=rec)
    return cfg, arrays


def pack_weights(W1, b1, W2, b2, W3, b3, Wl, bl, Wo, bo):
    """Pack the (replicated) weights into device layouts."""
    def to16(a):
        return np.asarray(a, np.float32).astype(BF16_NP)

    w1 = np.zeros((128, HIDDEN), BF16_NP)
    w1[:N_FEAT] = to16(W1)                                     # [64,256] pad K
    w2 = np.asarray(W2, np.float32).reshape(2, 128, HIDDEN)
    w2 = np.concatenate([to16(w2[0]), to16(w2[1])], axis=1)    # [128, 512]
    w3 = np.asarray(W3, np.float32).reshape(2, 128, HIDDEN)
    w3 = np.concatenate([to16(w3[0]), to16(w3[1])], axis=1)
    wl = np.asarray(Wl, np.float32).reshape(2, 128, 128)
    wl = np.concatenate([to16(wl[0]), to16(wl[1])], axis=1)    # [128, 256]
    wo = to16(np.asarray(Wo, np.float32).reshape(128, 1))      # [128, 1]
    b123 = np.stack([np.asarray(b, np.float32) * g
                     for b, g in zip((b1, b2, b3), GAINS)])
    b123 = b123.astype(BF16_NP).reshape(1, 3 * HIDDEN)         # [1, 768]
    bl_a = np.asarray(bl, np.float32).reshape(128, 1)          # [128, 1]
    bo_a = np.asarray(bo, np.float32).reshape(1, 1)
    return dict(w1=w1, w2=w2, w3=w3, wl=wl, wo=wo, b123=b123,
                bl=bl_a, bo=bo_a)


# -------------------------------------------------------------- bass kernel --

def build(cfg):
    """Build the SPMD Bass program (same graph on all 8 cores)."""
    T_half = cfg["T_half"]
    gw = cfg["gw"]
    calls = cfg["calls"]
    seg = cfg["seg"]
    T_tiles = 2 * T_half
    n_c = T_tiles * 128
    nh = T_half * 128
    t_kt = int(sum(a + b for a, b in gw))
    n_groups = (T_tiles + GS - 1) // GS

    nc = bacc.Bacc(None, target_bir_lowering=False,
                   dynamic_dma_scratch_size=49152,
                   num_swdge_queues=4)
    P = {}
    P["xT"] = nc.declare_dram_parameter("xT", [N_FEAT, n_c], BF16, False)
    P["disg"] = nc.declare_dram_parameter("disg", [128, 3 * T_tiles], F32, False)
    P["disu"] = nc.declare_dram_parameter("disu", [128, 3 * T_tiles], F32, False)
    P["invdis"] = nc.declare_dram_parameter("invdis", [1, n_c], BF16, False)
    P["batchloc"] = nc.declare_dram_parameter("batchloc", [128, T_tiles], F32, False)
    P["gidx"] = nc.declare_dram_parameter("gidx", [128, t_kt * 8], I16, False)
    selh_d = nc.declare_dram_parameter("selh", [128, t_kt * 128], FP8, False)
    P["rec"] = nc.declare_dram_parameter("rec", [128, GPC], F32, False)
    P["w1"] = nc.declare_dram_parameter("w1", [128, HIDDEN], BF16, False)
    P["w2"] = nc.declare_dram_parameter("w2", [128, 2 * HIDDEN], BF16, False)
    P["w3"] = nc.declare_dram_parameter("w3", [128, 2 * HIDDEN], BF16, False)
    P["wl"] = nc.declare_dram_parameter("wl", [128, 256], BF16, False)
    P["wo"] = nc.declare_dram_parameter("wo", [128, 1], BF16, False)
    P["b123"] = nc.declare_dram_parameter("b123", [1, 3 * HIDDEN], BF16, False)
    P["bl"] = nc.declare_dram_parameter("bl", [128, 1], F32, False)
    P["bo"] = nc.declare_dram_parameter("bo", [1, 1], F32, False)
    out_ext = nc.declare_dram_parameter("out", [1, GPC], F32, True)

    rg = [list(range(N_CORES))]
    AF = mybir.ActivationFunctionType
    OP = mybir.AluOpType

    with tile.TileContext(nc) as tc:
        with (
            tc.tile_pool(name="const", bufs=1) as cpool,
            tc.tile_pool(name="big", bufs=1) as bigpool,
            tc.tile_pool(name="work", bufs=3) as wpool,
            tc.tile_pool(name="gath", bufs=3) as gpool,
            tc.tile_pool(name="sel", bufs=2) as spool,
            tc.tile_pool(name="ps", bufs=2, space="PSUM") as pspool,
            tc.tile_pool(name="psa", bufs=3, space="PSUM") as papool,
            tc.tile_pool(name="pspool", bufs=1, space="PSUM") as ppool,
            tc.tile_pool(name="dram", bufs=2, space="DRAM") as dpool,
        ):
            # ---- constants / parameter loads (once) ----
            iota_t = cpool.tile([128, 128], F32)
            nc.gpsimd.iota(iota_t[:], pattern=[[1, 128]], base=0,
                           channel_multiplier=0,
                           allow_small_or_imprecise_dtypes=True)
            ident = cpool.tile([128, 128], BF16)
            make_identity(nc, ident[:])
            ident8 = cpool.tile([128, 128], GDT)
            nc.vector.tensor_copy(ident8[:], ident[:])

            def load(name, shape, dt):
                t = cpool.tile(list(shape), dt, name=f"sb_{name}")
                nc.sync.dma_start(out=t[:], in_=P[name][:, :])
                return t

            xT_sb = bigpool.tile([N_FEAT, n_c], BF16)
            nc.sync.dma_start(out=xT_sb[:], in_=P["xT"][:, :])
            disg_sb = load("disg", (128, 3 * T_tiles), F32)
            disu_sb = load("disu", (128, 3 * T_tiles), F32)
            invdis_sb = load("invdis", (1, n_c), BF16)
            batchloc_sb = load("batchloc", (128, T_tiles), F32)
            gidx_sb = bigpool.tile([128, t_kt * 8], I16)
            nc.sync.dma_start(out=gidx_sb[:], in_=P["gidx"][:, :])
            rec_sb = load("rec", (128, GPC), F32)
            w1_sb = load("w1", (128, HIDDEN), BF16)
            w2_sb = load("w2", (128, 2 * HIDDEN), BF16)
            w3_sb = load("w3", (128, 2 * HIDDEN), BF16)
            wl_sb = load("wl", (128, 256), BF16)
            wo_sb = load("wo", (128, 1), BF16)
            b123_sb = load("b123", (1, 3 * HIDDEN), BF16)
            bl_sb = load("bl", (128, 1), F32)
            bo_sb = load("bo", (1, 1), F32)

            # persistent transposed activations for the dense matmuls, and
            # the SBUF-resident z store (self-loop source)
            hT0 = bigpool.tile([128, n_c], BF16)
            hT1 = bigpool.tile([128, n_c], BF16)
            zstore = bigpool.tile([128, T_tiles * HIDDEN], GDT)

            pool_ps = [None, None]      # [chunk] psum tiles for poolT, per win
            out_sb = cpool.tile([1, GPC], F32)
            qno = [0]

            def emit_dense(ln, t, zloc_n):
                """z_{ln} for tile t -> zstore + DRAM table (pre-AllGather)."""
                sl = slice(t * 128, (t + 1) * 128)
                w_sb = (w1_sb, w2_sb, w3_sb)[ln]
                psz = pspool.tile([128, HIDDEN], F32, tag="mm", name="psz")
                if ln == 0:
                    nc.tensor.matmul(psz[:], lhsT=xT_sb[:, sl],
                                     rhs=w_sb[:N_FEAT, :HIDDEN],
                                     start=True, stop=True)
                else:
                    nc.tensor.matmul(psz[:], lhsT=hT0[:, sl],
                                     rhs=w_sb[:, 0:HIDDEN],
                                     start=True, stop=False)
                    nc.tensor.matmul(psz[:], lhsT=hT1[:, sl],
                                     rhs=w_sb[:, HIDDEN:2 * HIDDEN],
                                     start=False, stop=True)
                zsl = zstore[:, t * HIDDEN:(t + 1) * HIDDEN]
                nc.scalar.activation(
                    zsl, psz[:], AF.Copy,
                    scale=disg_sb[:, ln * T_tiles + t:ln * T_tiles + t + 1])
                hsl = slice((t % T_half) * 128, (t % T_half + 1) * 128)
                nc.sync.dma_start(out=zloc_n[t // T_half][hsl, :], in_=zsl)

            def alloc_zloc():
                return [dpool.tile([nh, HIDDEN], GDT, tag=f"zloc{w}",
                                   name=f"zloc{w}") for w in range(2)]

            def emit_ag(w, zloc_n, zfull_n):
                zfull_n[w] = dpool.tile([N_CORES * nh, HIDDEN], GDT,
                                        tag=f"zfull{w}", name=f"zfull{w}",
                                        addr_space="Shared")
                nc.gpsimd.collective_compute(
                    "AllGather", OP.bypass, replica_groups=rg,
                    ins=[zloc_n[w].opt()], outs=[zfull_n[w].opt()])

            def emit_gather(g, w, zfull_c):
                c0, nk = calls[2 * g + w]
                if nk == 0:
                    return None
                gt = gpool.tile([128, nk * HIDDEN], GDT, tag=f"gath{w}",
                                name=f"gath{w}")
                gv = gt[:].rearrange("p (k h) -> p k h", h=HIDDEN)
                nidx = nk * 128
                nc.gpsimd.dma_gather(
                    out_ap=gv, in_ap=zfull_c[w][:, :],
                    idxs_ap=gidx_sb[:, c0 * 8:(c0 + nk) * 8],
                    num_idxs=nidx, num_idxs_reg=nidx,
                    elem_size=HIDDEN, single_packet=False,
                    queue_num=qno[0] % 4)
                qno[0] += 1
                return (gt, c0)

            def emit_sel(g):
                c0 = calls[2 * g][0]
                nksum = calls[2 * g][1] + calls[2 * g + 1][1]
                sel_sb = spool.tile([128, nksum * 128], GDT, tag="sel",
                                    name="sel_sb")
                nc.sync.dma_start(
                    out=sel_sb[:],
                    in_=selh_d[:, c0 * 128:(c0 + nksum) * 128])
                return sel_sb

            # ---- prologue: z1 tables from x, AllGathers interleaved ----
            zloc_n = alloc_zloc()
            zfull_cur = [None, None]
            for t in range(T_half):
                emit_dense(0, t, zloc_n)
            emit_ag(0, zloc_n, zfull_cur)
            for t in range(T_half, T_tiles):
                emit_dense(0, t, zloc_n)
            emit_ag(1, zloc_n, zfull_cur)

            for layer in range(3):
                # software-pipelined gathers: A calls for the first two
                # groups go first so the gpsimd stream is not blocked on
                # the (just-triggered) B-table AllGather
                pend_g = {}
                pend_s = {}
                pend_g[(0, 0)] = emit_gather(0, 0, zfull_cur)
                if n_groups > 1:
                    pend_g[(1, 0)] = emit_gather(1, 0, zfull_cur)
                pend_g[(0, 1)] = emit_gather(0, 1, zfull_cur)
                pend_s[0] = emit_sel(0)
                if n_groups > 1:
                    pend_g[(1, 1)] = emit_gather(1, 1, zfull_cur)
                    pend_s[1] = emit_sel(1)

                ag_done = [False, False]
                zloc_nn = None
                if layer < 2:
                    zloc_nn = alloc_zloc()
                zfull_next = [None, None]

                b_row = b123_sb[0:1, layer * HIDDEN:(layer + 1) * HIDDEN]
                for g in range(n_groups):
                    ts = range(g * GS, min((g + 1) * GS, T_tiles))
                    if g + 2 < n_groups:
                        pend_g[(g + 2, 0)] = emit_gather(g + 2, 0, zfull_cur)
                        pend_g[(g + 2, 1)] = emit_gather(g + 2, 1, zfull_cur)
                        pend_s[g + 2] = emit_sel(g + 2)
                    gath = [pend_g.pop((g, 0)), pend_g.pop((g, 1))]
                    sel_sb = pend_s.pop(g)
                    c0 = calls[2 * g][0]

                    for t in ts:
                        sl = slice(t * 128, (t + 1) * 128)
                        psa = papool.tile([128, HIDDEN], F32, tag="psa")
                        # bias as outer product (1/dis)[dst] x b -> exact norm
                        nc.tensor.matmul(
                            psa[:], lhsT=invdis_sb[0:1, sl], rhs=b_row,
                            start=True, stop=False)
                        # self-loop term: agg[v] += z[v], from the z store
                        nc.tensor.matmul(
                            psa[:], lhsT=ident8[:],
                            rhs=zstore[:, t * HIDDEN:(t + 1) * HIDDEN],
                            start=False,
                            stop=(gw[t][0] + gw[t][1] == 0))
                        for w in range(2):
                            gwt = gw[t][w]
                            if gwt == 0:
                                continue
                            gt, gc0 = gath[w]
                            off = seg[t][w] - gc0
                            last = (w == 1) or gw[t][1] == 0
                            for j in range(gwt):
                                nc.tensor.matmul(
                                    psa[:],
                                    lhsT=sel_sb[:, (seg[t][w] - c0 + j) * 128:
                                                (seg[t][w] - c0 + j + 1) * 128],
                                    rhs=gt[:, (off + j) * HIDDEN:
                                           (off + j + 1) * HIDDEN],
                                    start=False,
                                    stop=(last and j == gwt - 1))
                        # h = relu(dis * agg + b)
                        h_sb = wpool.tile([128, HIDDEN], BF16, tag="h")
                        nc.scalar.activation(
                            h_sb[:], psa[:], AF.Relu,
                            scale=disu_sb[:, layer * T_tiles + t:
                                          layer * T_tiles + t + 1])
                        if layer < 2:
                            for cch in range(2):
                                pst = pspool.tile([128, 128], BF16, tag="mm")
                                nc.tensor.transpose(
                                    pst[:], h_sb[:, cch * 128:(cch + 1) * 128],
                                    ident[:])
                                hT = (hT0, hT1)[cch]
                                nc.vector.tensor_copy(hT[:, sl], pst[:])
                            # next layer's z for this tile rides along, so
                            # the AllGathers can overlap this layer's tail
                            emit_dense(layer + 1, t, zloc_nn)
                        else:
                            win = t // T_half
                            first = (t % T_half) == 0
                            last = (t % T_half) == T_half - 1
                            if first:
                                pool_ps[0] = ppool.tile([128, 128], F32,
                                                        name="poolT0",
                                                        tag="poolT0", bufs=1)
                                pool_ps[1] = ppool.tile([128, 128], F32,
                                                        name="poolT1",
                                                        tag="poolT1", bufs=1)
                            selp = spool.tile([128, 128], BF16, tag="selp")
                            nc.vector.tensor_tensor(
                                selp[:],
                                batchloc_sb[:, t:t + 1].to_broadcast([128, 128]),
                                iota_t[:], op=OP.is_equal)
                            for cch in range(2):
                                nc.tensor.matmul(
                                    pool_ps[cch][:],
                                    lhsT=h_sb[:, cch * 128:(cch + 1) * 128],
                                    rhs=selp[:], start=first, stop=last)
                            if last:
                                # ---- head for this window of 128 graphs ----
                                rrow = rec_sb[:, win * GPW:(win + 1) * GPW]
                                psu = papool.tile([128, GPW], F32, tag="head",
                                                  bufs=1)
                                for cch in range(2):
                                    gT = wpool.tile([128, GPW], BF16,
                                                    tag="gT")
                                    nc.vector.tensor_tensor(
                                        gT[:], pool_ps[cch][:, :GPW],
                                        rrow, op=OP.mult)
                                    nc.tensor.matmul(
                                        psu[:],
                                        lhsT=wl_sb[:, cch * 128:(cch + 1) * 128],
                                        rhs=gT[:], start=(cch == 0),
                                        stop=(cch == 1))
                                uT = wpool.tile([128, GPW], BF16, tag="uT")
                                nc.scalar.activation(uT[:], psu[:], AF.Relu,
                                                     bias=bl_sb[:, 0:1])
                                pso = papool.tile([1, GPW], F32, tag="head",
                                                  bufs=1)
                                nc.tensor.matmul(pso[:], lhsT=wo_sb[:, 0:1],
                                                 rhs=uT[:], start=True,
                                                 stop=True)
                                nc.vector.tensor_scalar(
                                    out_sb[0:1, win * GPW:(win + 1) * GPW],
                                    pso[:], bo_sb[0:1, 0:1], None, op0=OP.add)
                    if layer < 2:
                        if not ag_done[0] and (g + 1) * GS >= T_half:
                            emit_ag(0, zloc_nn, zfull_next)
                            ag_done[0] = True
                        if g == n_groups - 1:
                            emit_ag(1, zloc_nn, zfull_next)
                zfull_cur = zfull_next
            nc.sync.dma_start(out=out_ext[:, :], in_=out_sb[:])
    nc.finalize()
    return nc


# ------------------------------------------------------------------ runner --

_CACHE = {}


def _get_program(cfg):
    key = (cfg["T_half"], cfg["gw"])
    if key not in _CACHE:
        _CACHE[key] = build(cfg)
    return _CACHE[key]


def kernel(x, edge_index, batch, W1, b1, W2, b2, W3, b3, Wl, bl, Wo, bo):
    from concourse.bass_utils import run_bass_kernel_spmd

    cfg, arrays = preprocess(x, edge_index, batch)
    wts = pack_weights(W1, b1, W2, b2, W3, b3, Wl, bl, Wo, bo)
    nc = _get_program(cfg)

    in_maps = []
    for c in range(N_CORES):
        m = {k: np.ascontiguousarray(v[c]) for k, v in arrays.items()}
        m.update(wts)
        in_maps.append(m)

    res = run_bass_kernel_spmd(nc, in_maps, core_ids=list(range(N_CORES)))
    outs = res.results
    out = np.concatenate([outs[c]["out"].reshape(GPC) for c in range(N_CORES)])
    return out.reshape(N_GRAPHS, 1).astype(np.float32)


# revision 24
# speedup vs baseline: 1.2207x; 1.1416x over previous
"""Distributed 3-layer GCN (AqSolModel) on 8 TRN2 NeuronCores.

Strategy
--------
Nodes are partitioned by graph id (2048 graphs -> 256 graphs/core, nodes of a
graph never cross cores, so the segment-mean pool is core-local).  Per layer:

  z = (h @ W) scaled per-row by G_l*dis (dis=1/sqrt(deg); G_l is a per-layer
  gain that keeps fp8 values in normal range), stored as one fp8 row-table
  per half-window (A = tiles [0,T_half), B = rest) so table row ids fit
  int16; AllGather both tables across the 8 cores; per GROUP of 4 dst tiles,
  two dma_gather calls (one per source table, ~4.4k rows each, cycled over
  the 4 SWDGE queues so descriptor generation overlaps across Q7 core
  pairs) fetch the group's in-edge source rows; host-precomputed one-hot
  sel matrices stream from DRAM and PE segment-sums the gathered rows per
  dst tile; the self-loop term is an identity matmul from the SBUF-resident
  z store, and h = relu(dis/G_l * agg + b) is one ACT op (bias folded in as
  a K=1 outer-product matmul with the sqrt(deg) row, so GCN's symmetric
  norm comes out exactly).  The segment-mean pool + MLP head run per-core
  in a transposed layout (graphs never cross cores).
"""

import sys
import numpy as np

sys.path.insert(0, "/opt/trn_rl_repo")

import ml_dtypes

import concourse.bass as bass
import concourse.bacc as bacc
import concourse.mybir as mybir
import concourse.tile as tile
from concourse.masks import make_identity

N_NODES = 50000
N_EDGES = 800000
N_GRAPHS = 2048
N_FEAT = 64
HIDDEN = 256
N_CORES = 8
GPC = N_GRAPHS // N_CORES          # graphs per core (256)
GPW = GPC // 2                     # graphs per window (128)
GS = 4                             # dst tiles per gather group

F32 = mybir.dt.float32
BF16 = mybir.dt.bfloat16
FP8 = mybir.dt.float8e4
GDT = FP8            # gather-table dtype (z table, gathered rows, sel)
XG = 16.0            # fp8 gain of the layer-0 (dis*x) gather table
GAINS = (XG, 1024.0, 8192.0)     # per-layer fp8 dynamic-range gains
I32 = mybir.dt.int32
I16 = mybir.dt.int16
BF16_NP = ml_dtypes.bfloat16
FP8_NP = ml_dtypes.float8_e4m3


# ---------------------------------------------------------------- host side --

def preprocess(x, edge_index, batch):
    """Shard the graph across cores.  Returns (cfg, per-core input arrays)."""
    x = np.asarray(x, np.float32)
    src_g = np.asarray(edge_index[0], np.int64)
    dst_g = np.asarray(edge_index[1], np.int64)
    batch = np.asarray(batch, np.int64)

    # node -> core / half-window, contiguous because batch is sorted
    gsplit = np.searchsorted(batch, np.arange(0, N_GRAPHS + 1, GPW))  # 17 cuts
    half_cnt = np.diff(gsplit)                       # nodes per (core, half)
    T_half = int(np.max((half_cnt + 127) // 128))
    T_tiles = 2 * T_half
    n_c = T_tiles * 128                              # node slots per core
    nh = T_half * 128                                # node slots per window
    assert N_CORES * nh < 2 ** 15                    # table row ids fit int16

    # slot of each node inside its core
    core_of = np.repeat(np.arange(16) // 2, half_cnt)           # per node
    half_of = np.repeat(np.arange(16) % 2, half_cnt)
    rank_in_half = np.arange(N_NODES) - np.repeat(gsplit[:-1], half_cnt)
    slot = half_of * nh + rank_in_half

    # degree (in-degree + self loop) and norm factors
    deg = np.bincount(dst_g, minlength=N_NODES).astype(np.float64) + 1.0
    dis = (1.0 / np.sqrt(deg)).astype(np.float32)
    invdis = np.sqrt(deg).astype(np.float32)

    # real edges only; the self-loop term (z[v] into agg[v]) is applied on
    # device as an identity matmul from the SBUF-resident z store
    e_core = core_of[dst_g]
    e_tile = slot[dst_g] // 128
    e_local = slot[dst_g] % 128
    # src window: table A rows are window-0 slots, table B window-1
    e_win = half_of[src_g]                                       # 0=A, 1=B
    e_row = (core_of[src_g] * nh + (slot[src_g] % nh)).astype(np.int64)

    # sort edges by (dst core, dst tile, src window, src row)
    order = np.lexsort((e_row, e_win, e_tile, e_core))
    e_core, e_tile, e_local, e_win, e_row = (
        a[order] for a in (e_core, e_tile, e_local, e_win, e_row))

    counts = np.zeros((N_CORES, T_tiles, 2), np.int64)
    np.add.at(counts, (e_core, e_tile, e_win), 1)
    gw = ((counts.max(axis=0) + 127) // 128).astype(np.int64)    # [T,2] ktiles

    # global K-tile enumeration: per group of GS tiles, first all A-window
    # segments (one gather call), then all B-window segments (second call).
    n_groups = (T_tiles + GS - 1) // GS
    seg_off = np.zeros((T_tiles, 2), np.int64)   # K-tile col of each (t,w)
    call_cols = []                               # (start, ncols) per call
    kcur = 0
    for g in range(n_groups):
        ts = range(g * GS, min((g + 1) * GS, T_tiles))
        for w in range(2):
            c0 = kcur
            for t in ts:
                seg_off[t, w] = kcur
                kcur += int(gw[t, w])
            call_cols.append((c0, kcur - c0))
    t_kt = kcur

    # pack wrapped int16 gather indices [128, t_kt*8] and host-built one-hot
    # sel tables [128, t_kt*128] (value 1 at [stream slot, ktile*128+dst])
    gidx = np.zeros((N_CORES, 16, t_kt * 8), np.int16)
    selh = np.zeros((N_CORES, 128, t_kt * 128), np.uint8)
    bucket = (e_core * T_tiles + e_tile) * 2 + e_win
    bb = np.zeros(N_CORES * T_tiles * 2 + 1, np.int64)
    np.add.at(bb, bucket + 1, 1)
    bb = np.cumsum(bb)
    pos = np.arange(len(e_row)) - bb[bucket]     # rank within (t, w) bucket
    # stream position of this edge: seg base + rank
    spos = seg_off[e_tile, e_win] * 128 + pos
    gidx[e_core, spos % 16, spos // 16] = e_row.astype(np.int16)
    selh[e_core, spos % 128, (spos // 128) * 128 + e_local] = 1
    selh = selh.astype(FP8_NP)

    # layer-0 gather tables: full (replicated) fp8 XG*dis*x rows per window,
    # padded to 256 cols so elem_size_bytes % 256 == 0; plus the core-local
    # rows for the self-loop term
    xdis = (x * (dis * XG)[:, None]).astype(FP8_NP)              # [N, 64]
    xtab = np.zeros((2, N_CORES * nh, HIDDEN), FP8_NP)
    xtab[half_of, core_of * nh + slot % nh, :N_FEAT] = xdis
    xloc = np.zeros((N_CORES, 128, T_tiles * N_FEAT), FP8_NP)
    tcol = (slot // 128)[:, None] * N_FEAT + np.arange(N_FEAT)   # [N, 64]
    xloc[core_of[:, None], (slot % 128)[:, None], tcol] = xdis

    # per-core node-side arrays
    dis_a = np.zeros((N_CORES, 128, T_tiles), np.float32)
    invdis_a = np.zeros((N_CORES, 1, n_c), BF16_NP)
    batchloc = np.full((N_CORES, 128, T_tiles), -1, np.float32)
    dis_a[core_of, slot % 128, slot // 128] = dis
    invdis_a[core_of, 0, slot] = invdis.astype(BF16_NP)
    batchloc[core_of, slot % 128, slot // 128] = (batch % GPW).astype(np.float32)

    # per-graph 1/max(count,1), [1, 256] per core (window-major)
    gcnt = np.bincount(batch, minlength=N_GRAPHS).astype(np.float32)
    rec = (1.0 / np.maximum(gcnt, 1.0)).reshape(N_CORES, 1, GPC)
    rec = np.ascontiguousarray(np.broadcast_to(rec, (N_CORES, 128, GPC)))

    # per-layer fp8 gain: z stored as G_l * z, undone in the epilogue
    disg = np.concatenate([dis_a * g for g in GAINS], axis=2)     # z-store
    disu = np.concatenate([dis_a / g for g in GAINS], axis=2)     # epilogue

    cfg = dict(T_half=T_half,
               gw=tuple((int(a), int(b)) for a, b in gw),
               calls=tuple((int(a), int(b)) for a, b in call_cols),
               seg=tuple((int(a), int(b)) for a, b in seg_off))
    gidx = np.tile(gidx, (1, 8, 1))
    arrays = dict(disg=disg, disu=disu, invdis=invdis_a,
                  batchloc=batchloc, gidx=gidx, selh=selh, rec=rec,
                  xloc=xloc,
                  xtab0=np.broadcast_to(xtab[0], (N_CORES,) + xtab[0].shape),
                  xtab1=np.broadcast_to(xtab[1], (N_CORES,) + xtab[1].shape))
    return cfg, arrays


def pack_weights(W1, b1, W2, b2, W3, b3, Wl, bl, Wo, bo):
    """Pack the (replicated) weights into device layouts."""
    def to16(a):
        return np.asarray(a, np.float32).astype(BF16_NP)

    w1 = np.zeros((128, HIDDEN), BF16_NP)
    w1[:N_FEAT] = to16(W1)                                     # [64,256] pad K
    w2 = np.asarray(W2, np.float32).reshape(2, 128, HIDDEN)
    w2 = np.concatenate([to16(w2[0]), to16(w2[1])], axis=1)    # [128, 512]
    w3 = np.asarray(W3, np.float32).reshape(2, 128, HIDDEN)
    w3 = np.concatenate([to16(w3[0]), to16(w3[1])], axis=1)
    wl = np.asarray(Wl, np.float32).reshape(2, 128, 128)
    wl = np.concatenate([to16(wl[0]), to16(wl[1])], axis=1)    # [128, 256]
    wo = to16(np.asarray(Wo, np.float32).reshape(128, 1))      # [128, 1]
    # b1 is applied plain at the psz stage of layer 0 (after W1); b2/b3 ride
    # the gained aggregation epilogue
    b123 = np.stack([np.asarray(b, np.float32) * g
                     for b, g in zip((b1, b2, b3),
                                     (1.0, GAINS[1], GAINS[2]))])
    b123 = b123.astype(BF16_NP).reshape(1, 3 * HIDDEN)         # [1, 768]
    bl_a = np.asarray(bl, np.float32).reshape(128, 1)          # [128, 1]
    bo_a = np.asarray(bo, np.float32).reshape(1, 1)
    return dict(w1=w1, w2=w2, w3=w3, wl=wl, wo=wo, b123=b123,
                bl=bl_a, bo=bo_a)


# -------------------------------------------------------------- bass kernel --

def build(cfg):
    """Build the SPMD Bass program (same graph on all 8 cores)."""
    T_half = cfg["T_half"]
    gw = cfg["gw"]
    calls = cfg["calls"]
    seg = cfg["seg"]
    T_tiles = 2 * T_half
    n_c = T_tiles * 128
    nh = T_half * 128
    t_kt = int(sum(a + b for a, b in gw))
    n_groups = (T_tiles + GS - 1) // GS

    nc = bacc.Bacc(None, target_bir_lowering=False,
                   dynamic_dma_scratch_size=49152,
                   num_swdge_queues=4)
    P = {}
    P["xloc"] = nc.declare_dram_parameter("xloc", [128, T_tiles * N_FEAT],
                                          FP8, False)
    xtab = [nc.declare_dram_parameter(f"xtab{w}", [N_CORES * nh, HIDDEN],
                                      FP8, False) for w in range(2)]
    P["disg"] = nc.declare_dram_parameter("disg", [128, 3 * T_tiles], F32, False)
    P["disu"] = nc.declare_dram_parameter("disu", [128, 3 * T_tiles], F32, False)
    P["invdis"] = nc.declare_dram_parameter("invdis", [1, n_c], BF16, False)
    P["batchloc"] = nc.declare_dram_parameter("batchloc", [128, T_tiles], F32, False)
    P["gidx"] = nc.declare_dram_parameter("gidx", [128, t_kt * 8], I16, False)
    selh_d = nc.declare_dram_parameter("selh", [128, t_kt * 128], FP8, False)
    P["rec"] = nc.declare_dram_parameter("rec", [128, GPC], F32, False)
    P["w1"] = nc.declare_dram_parameter("w1", [128, HIDDEN], BF16, False)
    P["w2"] = nc.declare_dram_parameter("w2", [128, 2 * HIDDEN], BF16, False)
    P["w3"] = nc.declare_dram_parameter("w3", [128, 2 * HIDDEN], BF16, False)
    P["wl"] = nc.declare_dram_parameter("wl", [128, 256], BF16, False)
    P["wo"] = nc.declare_dram_parameter("wo", [128, 1], BF16, False)
    P["b123"] = nc.declare_dram_parameter("b123", [1, 3 * HIDDEN], BF16, False)
    P["bl"] = nc.declare_dram_parameter("bl", [128, 1], F32, False)
    P["bo"] = nc.declare_dram_parameter("bo", [1, 1], F32, False)
    out_ext = nc.declare_dram_parameter("out", [1, GPC], F32, True)

    rg = [list(range(N_CORES))]
    AF = mybir.ActivationFunctionType
    OP = mybir.AluOpType

    with tile.TileContext(nc) as tc:
        with (
            tc.tile_pool(name="const", bufs=1) as cpool,
            tc.tile_pool(name="big", bufs=1) as bigpool,
            tc.tile_pool(name="work", bufs=3) as wpool,
            tc.tile_pool(name="gath", bufs=3) as gpool,
            tc.tile_pool(name="sel", bufs=2) as spool,
            tc.tile_pool(name="ps", bufs=2, space="PSUM") as pspool,
            tc.tile_pool(name="psa", bufs=3, space="PSUM") as papool,
            tc.tile_pool(name="pspool", bufs=1, space="PSUM") as ppool,
            tc.tile_pool(name="dram", bufs=2, space="DRAM") as dpool,
        ):
            # ---- constants / parameter loads (once) ----
            iota_t = cpool.tile([128, 128], F32)
            nc.gpsimd.iota(iota_t[:], pattern=[[1, 128]], base=0,
                           channel_multiplier=0,
                           allow_small_or_imprecise_dtypes=True)
            ident = cpool.tile([128, 128], BF16)
            make_identity(nc, ident[:])
            ident8 = cpool.tile([128, 128], GDT)
            nc.vector.tensor_copy(ident8[:], ident[:])
            ones1 = cpool.tile([1, 128], BF16)
            nc.vector.memset(ones1[:], 1.0)

            def load(name, shape, dt):
                t = cpool.tile(list(shape), dt, name=f"sb_{name}")
                nc.sync.dma_start(out=t[:], in_=P[name][:, :])
                return t

            xloc_sb = bigpool.tile([128, T_tiles * N_FEAT], GDT)
            nc.sync.dma_start(out=xloc_sb[:], in_=P["xloc"][:, :])
            disg_sb = load("disg", (128, 3 * T_tiles), F32)
            disu_sb = load("disu", (128, 3 * T_tiles), F32)
            invdis_sb = load("invdis", (1, n_c), BF16)
            batchloc_sb = load("batchloc", (128, T_tiles), F32)
            gidx_sb = bigpool.tile([128, t_kt * 8], I16)
            nc.sync.dma_start(out=gidx_sb[:], in_=P["gidx"][:, :])
            rec_sb = load("rec", (128, GPC), F32)
            w1_sb = load("w1", (128, HIDDEN), BF16)
            w2_sb = load("w2", (128, 2 * HIDDEN), BF16)
            w3_sb = load("w3", (128, 2 * HIDDEN), BF16)
            wl_sb = load("wl", (128, 256), BF16)
            wo_sb = load("wo", (128, 1), BF16)
            b123_sb = load("b123", (1, 3 * HIDDEN), BF16)
            bl_sb = load("bl", (128, 1), F32)
            bo_sb = load("bo", (1, 1), F32)

            # persistent transposed activations for the dense matmuls, and
            # the SBUF-resident z store (self-loop source)
            hT0 = bigpool.tile([128, n_c], BF16)
            hT1 = bigpool.tile([128, n_c], BF16)
            zstore = bigpool.tile([128, T_tiles * HIDDEN], GDT)

            pool_ps = [None, None]      # [chunk] psum tiles for poolT, per win
            out_sb = cpool.tile([1, GPC], F32)
            qno = [0]

            def emit_dense(ln, t, zloc_n):
                """z_{ln} for tile t -> zstore + DRAM table (pre-AllGather)."""
                sl = slice(t * 128, (t + 1) * 128)
                w_sb = (w1_sb, w2_sb, w3_sb)[ln]
                psz = pspool.tile([128, HIDDEN], F32, tag="mm", name="psz")
                nc.tensor.matmul(psz[:], lhsT=hT0[:, sl],
                                 rhs=w_sb[:, 0:HIDDEN],
                                 start=True, stop=False)
                nc.tensor.matmul(psz[:], lhsT=hT1[:, sl],
                                 rhs=w_sb[:, HIDDEN:2 * HIDDEN],
                                 start=False, stop=True)
                zsl = zstore[:, t * HIDDEN:(t + 1) * HIDDEN]
                nc.scalar.activation(
                    zsl, psz[:], AF.Copy,
                    scale=disg_sb[:, ln * T_tiles + t:ln * T_tiles + t + 1])
                hsl = slice((t % T_half) * 128, (t % T_half + 1) * 128)
                nc.sync.dma_start(out=zloc_n[t // T_half][hsl, :], in_=zsl)

            def alloc_zloc():
                return [dpool.tile([nh, HIDDEN], GDT, tag=f"zloc{w}",
                                   name=f"zloc{w}") for w in range(2)]

            def emit_ag(w, zloc_n, zfull_n):
                zfull_n[w] = dpool.tile([N_CORES * nh, HIDDEN], GDT,
                                        tag=f"zfull{w}", name=f"zfull{w}",
                                        addr_space="Shared")
                nc.gpsimd.collective_compute(
                    "AllGather", OP.bypass, replica_groups=rg,
                    ins=[zloc_n[w].opt()], outs=[zfull_n[w].opt()])

            def emit_gather(g, w, zfull_c):
                c0, nk = calls[2 * g + w]
                if nk == 0:
                    return None
                gt = gpool.tile([128, nk * HIDDEN], GDT, tag=f"gath{w}",
                                name=f"gath{w}", bufs=(4 if w == 0 else 3))
                gv = gt[:].rearrange("p (k h) -> p k h", h=HIDDEN)
                nidx = nk * 128
                nc.gpsimd.dma_gather(
                    out_ap=gv, in_ap=zfull_c[w][:, :],
                    idxs_ap=gidx_sb[:, c0 * 8:(c0 + nk) * 8],
                    num_idxs=nidx, num_idxs_reg=nidx,
                    elem_size=HIDDEN, single_packet=False,
                    queue_num=qno[0] % 4)
                qno[0] += 1
                return (gt, c0)

            def emit_sel(g):
                c0 = calls[2 * g][0]
                nksum = calls[2 * g][1] + calls[2 * g + 1][1]
                sel_sb = spool.tile([128, nksum * 128], GDT, tag="sel",
                                    name="sel_sb", bufs=3)
                nc.sync.dma_start(
                    out=sel_sb[:],
                    in_=selh_d[:, c0 * 128:(c0 + nksum) * 128])
                return sel_sb

            # layer 0 gathers straight from the host-replicated x tables —
            # no dense prologue and no layer-0 AllGather
            zfull_cur = xtab

            for layer in range(3):
                # software-pipelined gathers: A calls for the first three
                # groups go first so the gpsimd stream is not blocked on
                # the (just-triggered) B-table AllGather
                pend_g = {}
                pend_s = {}
                for ga in range(min(3, n_groups)):
                    pend_g[(ga, 0)] = emit_gather(ga, 0, zfull_cur)
                pend_g[(0, 1)] = emit_gather(0, 1, zfull_cur)
                pend_s[0] = emit_sel(0)
                if n_groups > 1:
                    pend_g[(1, 1)] = emit_gather(1, 1, zfull_cur)
                    pend_s[1] = emit_sel(1)

                ag_done = [False, False]
                zloc_nn = None
                if layer < 2:
                    zloc_nn = alloc_zloc()
                zfull_next = [None, None]

                b_row = b123_sb[0:1, layer * HIDDEN:(layer + 1) * HIDDEN]
                for g in range(n_groups):
                    ts = range(g * GS, min((g + 1) * GS, T_tiles))
                    if g + 3 < n_groups:
                        pend_g[(g + 3, 0)] = emit_gather(g + 3, 0, zfull_cur)
                    if g + 2 < n_groups:
                        pend_g[(g + 2, 1)] = emit_gather(g + 2, 1, zfull_cur)
                        pend_s[g + 2] = emit_sel(g + 2)
                    gath = [pend_g.pop((g, 0)), pend_g.pop((g, 1))]
                    sel_sb = pend_s.pop(g)
                    c0 = calls[2 * g][0]

                    for t in ts:
                        sl = slice(t * 128, (t + 1) * 128)
                        agw = HIDDEN if layer else N_FEAT
                        psa = papool.tile([128, agw], F32, tag="psa")
                        if layer:
                            # bias as (1/dis)[dst] (x) b -> exact norm
                            nc.tensor.matmul(
                                psa[:], lhsT=invdis_sb[0:1, sl], rhs=b_row,
                                start=True, stop=False)
                        # self-loop term: agg[v] += z[v] (x[v] for layer 0)
                        nc.tensor.matmul(
                            psa[:], lhsT=ident8[:],
                            rhs=(zstore[:, t * HIDDEN:(t + 1) * HIDDEN]
                                 if layer else
                                 xloc_sb[:, t * N_FEAT:(t + 1) * N_FEAT]),
                            start=(layer == 0),
                            stop=(gw[t][0] + gw[t][1] == 0))
                        for w in range(2):
                            gwt = gw[t][w]
                            if gwt == 0:
                                continue
                            gt, gc0 = gath[w]
                            off = seg[t][w] - gc0
                            last = (w == 1) or gw[t][1] == 0
                            for j in range(gwt):
                                nc.tensor.matmul(
                                    psa[:],
                                    lhsT=sel_sb[:, (seg[t][w] - c0 + j) * 128:
                                                (seg[t][w] - c0 + j + 1) * 128],
                                    rhs=gt[:, (off + j) * HIDDEN:
                                           (off + j) * HIDDEN + agw],
                                    start=False,
                                    stop=(last and j == gwt - 1))
                        if layer == 0:
                            # u = dis * agg; h1 = relu(uT^T @ W1 + b1)
                            u_sb = wpool.tile([128, N_FEAT], BF16, tag="u")
                            nc.scalar.activation(
                                u_sb[:], psa[:], AF.Copy,
                                scale=disu_sb[:, t:t + 1])
                            psu0 = pspool.tile([N_FEAT, 128], BF16, tag="mm",
                                               name="psu0")
                            nc.tensor.transpose(psu0[:], u_sb[:], ident[:])
                            uT_sb = wpool.tile([N_FEAT, 128], BF16, tag="uT")
                            nc.vector.tensor_copy(uT_sb[:], psu0[:])
                            psz1 = papool.tile([128, HIDDEN], F32, tag="psa",
                                               name="psz1")
                            nc.tensor.matmul(psz1[:], lhsT=uT_sb[:],
                                             rhs=w1_sb[:N_FEAT, :],
                                             start=True, stop=False)
                            nc.tensor.matmul(psz1[:], lhsT=ones1[:],
                                             rhs=b_row, start=False, stop=True)
                            psa = psz1
                        # h = relu(dis * agg + b)   (layer 0: relu(z1 + b1))
                        h_sb = wpool.tile([128, HIDDEN], BF16, tag="h")
                        if layer:
                            nc.scalar.activation(
                                h_sb[:], psa[:], AF.Relu,
                                scale=disu_sb[:, layer * T_tiles + t:
                                              layer * T_tiles + t + 1])
                        else:
                            nc.scalar.activation(h_sb[:], psa[:], AF.Relu)
                        if layer < 2:
                            for cch in range(2):
                                pst = pspool.tile([128, 128], BF16, tag="mm")
                                nc.tensor.transpose(
                                    pst[:], h_sb[:, cch * 128:(cch + 1) * 128],
                                    ident[:])
                                hT = (hT0, hT1)[cch]
                                nc.vector.tensor_copy(hT[:, sl], pst[:])
                            # next layer's z for this tile rides along, so
                            # the AllGathers can overlap this layer's tail
                            emit_dense(layer + 1, t, zloc_nn)
                        else:
                            win = t // T_half
                            first = (t % T_half) == 0
                            last = (t % T_half) == T_half - 1
                            if first:
                                pool_ps[0] = ppool.tile([128, 128], F32,
                                                        name="poolT0",
                                                        tag="poolT0", bufs=1)
                                pool_ps[1] = ppool.tile([128, 128], F32,
                                                        name="poolT1",
                                                        tag="poolT1", bufs=1)
                            selp = spool.tile([128, 128], BF16, tag="selp")
                            nc.vector.tensor_tensor(
                                selp[:],
                                batchloc_sb[:, t:t + 1].to_broadcast([128, 128]),
                                iota_t[:], op=OP.is_equal)
                            for cch in range(2):
                                nc.tensor.matmul(
                                    pool_ps[cch][:],
                                    lhsT=h_sb[:, cch * 128:(cch + 1) * 128],
                                    rhs=selp[:], start=first, stop=last)
                            if last:
                                # ---- head for this window of 128 graphs ----
                                rrow = rec_sb[:, win * GPW:(win + 1) * GPW]
                                psu = papool.tile([128, GPW], F32, tag="head",
                                                  bufs=1)
                                for cch in range(2):
                                    gT = wpool.tile([128, GPW], BF16,
                                                    tag="gT")
                                    nc.vector.tensor_tensor(
                                        gT[:], pool_ps[cch][:, :GPW],
                                        rrow, op=OP.mult)
                                    nc.tensor.matmul(
                                        psu[:],
                                        lhsT=wl_sb[:, cch * 128:(cch + 1) * 128],
                                        rhs=gT[:], start=(cch == 0),
                                        stop=(cch == 1))
                                uT = wpool.tile([128, GPW], BF16, tag="uT")
                                nc.scalar.activation(uT[:], psu[:], AF.Relu,
                                                     bias=bl_sb[:, 0:1])
                                pso = papool.tile([1, GPW], F32, tag="head",
                                                  bufs=1)
                                nc.tensor.matmul(pso[:], lhsT=wo_sb[:, 0:1],
                                                 rhs=uT[:], start=True,
                                                 stop=True)
                                nc.vector.tensor_scalar(
                                    out_sb[0:1, win * GPW:(win + 1) * GPW],
                                    pso[:], bo_sb[0:1, 0:1], None, op0=OP.add)
                    if layer < 2:
                        if not ag_done[0] and (g + 1) * GS >= T_half:
                            emit_ag(0, zloc_nn, zfull_next)
                            ag_done[0] = True
                        if g == n_groups - 1:
                            emit_ag(1, zloc_nn, zfull_next)
                zfull_cur = zfull_next
            nc.sync.dma_start(out=out_ext[:, :], in_=out_sb[:])
    nc.finalize()
    return nc


# ------------------------------------------------------------------ runner --

_CACHE = {}


def _get_program(cfg):
    key = (cfg["T_half"], cfg["gw"])
    if key not in _CACHE:
        _CACHE[key] = build(cfg)
    return _CACHE[key]


def kernel(x, edge_index, batch, W1, b1, W2, b2, W3, b3, Wl, bl, Wo, bo):
    from concourse.bass_utils import run_bass_kernel_spmd

    cfg, arrays = preprocess(x, edge_index, batch)
    wts = pack_weights(W1, b1, W2, b2, W3, b3, Wl, bl, Wo, bo)
    nc = _get_program(cfg)

    in_maps = []
    for c in range(N_CORES):
        m = {k: np.ascontiguousarray(v[c]) for k, v in arrays.items()}
        m.update(wts)
        in_maps.append(m)

    res = run_bass_kernel_spmd(nc, in_maps, core_ids=list(range(N_CORES)))
    outs = res.results
    out = np.concatenate([outs[c]["out"].reshape(GPC) for c in range(N_CORES)])
    return out.reshape(N_GRAPHS, 1).astype(np.float32)


# revision 26
# speedup vs baseline: 1.2209x; 1.0002x over previous
"""Distributed 3-layer GCN (AqSolModel) on 8 TRN2 NeuronCores.

Strategy
--------
Nodes are partitioned by graph id (2048 graphs -> 256 graphs/core, nodes of a
graph never cross cores, so the segment-mean pool is core-local).  Per layer:

  z = (h @ W) scaled per-row by G_l*dis (dis=1/sqrt(deg); G_l is a per-layer
  gain that keeps fp8 values in normal range), stored as one fp8 row-table
  per half-window (A = tiles [0,T_half), B = rest) so table row ids fit
  int16; AllGather both tables across the 8 cores; per GROUP of 4 dst tiles,
  two dma_gather calls (one per source table, ~4.4k rows each, cycled over
  the 4 SWDGE queues so descriptor generation overlaps across Q7 core
  pairs) fetch the group's in-edge source rows; host-precomputed one-hot
  sel matrices stream from DRAM and PE segment-sums the gathered rows per
  dst tile; the self-loop term is an identity matmul from the SBUF-resident
  z store, and h = relu(dis/G_l * agg + b) is one ACT op (bias folded in as
  a K=1 outer-product matmul with the sqrt(deg) row, so GCN's symmetric
  norm comes out exactly).  The segment-mean pool + MLP head run per-core
  in a transposed layout (graphs never cross cores).
"""

import sys
import numpy as np

sys.path.insert(0, "/opt/trn_rl_repo")

import ml_dtypes

import concourse.bass as bass
import concourse.bacc as bacc
import concourse.mybir as mybir
import concourse.tile as tile
from concourse.masks import make_identity

N_NODES = 50000
N_EDGES = 800000
N_GRAPHS = 2048
N_FEAT = 64
HIDDEN = 256
N_CORES = 8
GPC = N_GRAPHS // N_CORES          # graphs per core (256)
GPW = GPC // 2                     # graphs per window (128)
GS = 4                             # dst tiles per gather group

F32 = mybir.dt.float32
BF16 = mybir.dt.bfloat16
FP8 = mybir.dt.float8e4
GDT = FP8            # gather-table dtype (z table, gathered rows, sel)
XG = 16.0            # fp8 gain of the layer-0 (dis*x) gather table
GAINS = (XG, 1024.0, 8192.0)     # per-layer fp8 dynamic-range gains
I32 = mybir.dt.int32
I16 = mybir.dt.int16
BF16_NP = ml_dtypes.bfloat16
FP8_NP = ml_dtypes.float8_e4m3


# ---------------------------------------------------------------- host side --

def preprocess(x, edge_index, batch):
    """Shard the graph across cores.  Returns (cfg, per-core input arrays)."""
    x = np.asarray(x, np.float32)
    src_g = np.asarray(edge_index[0], np.int64)
    dst_g = np.asarray(edge_index[1], np.int64)
    batch = np.asarray(batch, np.int64)

    # node -> core / half-window, contiguous because batch is sorted
    gsplit = np.searchsorted(batch, np.arange(0, N_GRAPHS + 1, GPW))  # 17 cuts
    half_cnt = np.diff(gsplit)                       # nodes per (core, half)
    T_half = int(np.max((half_cnt + 127) // 128))
    T_tiles = 2 * T_half
    n_c = T_tiles * 128                              # node slots per core
    nh = T_half * 128                                # node slots per window
    assert N_CORES * nh < 2 ** 15                    # table row ids fit int16

    # slot of each node inside its core
    core_of = np.repeat(np.arange(16) // 2, half_cnt)           # per node
    half_of = np.repeat(np.arange(16) % 2, half_cnt)
    rank_in_half = np.arange(N_NODES) - np.repeat(gsplit[:-1], half_cnt)
    slot = half_of * nh + rank_in_half

    # degree (in-degree + self loop) and norm factors
    deg = np.bincount(dst_g, minlength=N_NODES).astype(np.float64) + 1.0
    dis = (1.0 / np.sqrt(deg)).astype(np.float32)
    invdis = np.sqrt(deg).astype(np.float32)

    # real edges only; the self-loop term (z[v] into agg[v]) is applied on
    # device as an identity matmul from the SBUF-resident z store
    e_core = core_of[dst_g]
    e_tile = slot[dst_g] // 128
    e_local = slot[dst_g] % 128
    # src window: table A rows are window-0 slots, table B window-1
    e_win = half_of[src_g]                                       # 0=A, 1=B
    e_row = (core_of[src_g] * nh + (slot[src_g] % nh)).astype(np.int64)

    # sort edges by (dst core, dst tile, src window, src row)
    order = np.lexsort((e_row, e_win, e_tile, e_core))
    e_core, e_tile, e_local, e_win, e_row = (
        a[order] for a in (e_core, e_tile, e_local, e_win, e_row))

    counts = np.zeros((N_CORES, T_tiles, 2), np.int64)
    np.add.at(counts, (e_core, e_tile, e_win), 1)
    gw = ((counts.max(axis=0) + 127) // 128).astype(np.int64)    # [T,2] ktiles

    # global K-tile enumeration: per group of GS tiles, first all A-window
    # segments (one gather call), then all B-window segments (second call).
    n_groups = (T_tiles + GS - 1) // GS
    seg_off = np.zeros((T_tiles, 2), np.int64)   # K-tile col of each (t,w)
    call_cols = []                               # (start, ncols) per call
    kcur = 0
    for g in range(n_groups):
        ts = range(g * GS, min((g + 1) * GS, T_tiles))
        for w in range(2):
            c0 = kcur
            for t in ts:
                seg_off[t, w] = kcur
                kcur += int(gw[t, w])
            call_cols.append((c0, kcur - c0))
    t_kt = kcur

    # pack wrapped int16 gather indices [128, t_kt*8] and host-built one-hot
    # sel tables [128, t_kt*128] (value 1 at [stream slot, ktile*128+dst])
    gidx = np.zeros((N_CORES, 16, t_kt * 8), np.int16)
    selh = np.zeros((N_CORES, 128, t_kt * 128), np.uint8)
    bucket = (e_core * T_tiles + e_tile) * 2 + e_win
    bb = np.zeros(N_CORES * T_tiles * 2 + 1, np.int64)
    np.add.at(bb, bucket + 1, 1)
    bb = np.cumsum(bb)
    pos = np.arange(len(e_row)) - bb[bucket]     # rank within (t, w) bucket
    # stream position of this edge: seg base + rank
    spos = seg_off[e_tile, e_win] * 128 + pos
    gidx[e_core, spos % 16, spos // 16] = e_row.astype(np.int16)
    selh[e_core, spos % 128, (spos // 128) * 128 + e_local] = 1
    selh = selh.astype(FP8_NP)

    # layer-0 gather tables: full (replicated) fp8 XG*dis*x rows per window,
    # padded to 256 cols so elem_size_bytes % 256 == 0; plus the core-local
    # rows for the self-loop term
    xdis = (x * (dis * XG)[:, None]).astype(FP8_NP)              # [N, 64]
    xtab = np.zeros((2, N_CORES * nh, HIDDEN), FP8_NP)
    xtab[half_of, core_of * nh + slot % nh, :N_FEAT] = xdis
    xloc = np.zeros((N_CORES, 128, T_tiles * N_FEAT), FP8_NP)
    tcol = (slot // 128)[:, None] * N_FEAT + np.arange(N_FEAT)   # [N, 64]
    xloc[core_of[:, None], (slot % 128)[:, None], tcol] = xdis

    # per-core node-side arrays
    dis_a = np.zeros((N_CORES, 128, T_tiles), np.float32)
    invdis_a = np.zeros((N_CORES, 1, n_c), BF16_NP)
    batchloc = np.full((N_CORES, 128, T_tiles), -1, np.float32)
    dis_a[core_of, slot % 128, slot // 128] = dis
    invdis_a[core_of, 0, slot] = invdis.astype(BF16_NP)
    batchloc[core_of, slot % 128, slot // 128] = (batch % GPW).astype(np.float32)

    # per-graph 1/max(count,1), [1, 256] per core (window-major)
    gcnt = np.bincount(batch, minlength=N_GRAPHS).astype(np.float32)
    rec = (1.0 / np.maximum(gcnt, 1.0)).reshape(N_CORES, 1, GPC)
    rec = np.ascontiguousarray(np.broadcast_to(rec, (N_CORES, 128, GPC)))

    # per-layer fp8 gain: z stored as G_l * z, undone in the epilogue
    disg = np.concatenate([dis_a * g for g in GAINS], axis=2)     # z-store
    disu = np.concatenate([dis_a / g for g in GAINS], axis=2)     # epilogue

    cfg = dict(T_half=T_half,
               gw=tuple((int(a), int(b)) for a, b in gw),
               calls=tuple((int(a), int(b)) for a, b in call_cols),
               seg=tuple((int(a), int(b)) for a, b in seg_off))
    gidx = np.tile(gidx, (1, 8, 1))
    arrays = dict(disg=disg, disu=disu, invdis=invdis_a,
                  batchloc=batchloc, gidx=gidx, selh=selh, rec=rec,
                  xloc=xloc,
                  xtab0=np.broadcast_to(xtab[0], (N_CORES,) + xtab[0].shape),
                  xtab1=np.broadcast_to(xtab[1], (N_CORES,) + xtab[1].shape))
    return cfg, arrays


def pack_weights(W1, b1, W2, b2, W3, b3, Wl, bl, Wo, bo):
    """Pack the (replicated) weights into device layouts."""
    def to16(a):
        return np.asarray(a, np.float32).astype(BF16_NP)

    w1 = np.zeros((128, HIDDEN), BF16_NP)
    w1[:N_FEAT] = to16(W1)                                     # [64,256] pad K
    w2 = np.asarray(W2, np.float32).reshape(2, 128, HIDDEN)
    w2 = np.concatenate([to16(w2[0]), to16(w2[1])], axis=1)    # [128, 512]
    w3 = np.asarray(W3, np.float32).reshape(2, 128, HIDDEN)
    w3 = np.concatenate([to16(w3[0]), to16(w3[1])], axis=1)
    wl = np.asarray(Wl, np.float32).reshape(2, 128, 128)
    wl = np.concatenate([to16(wl[0]), to16(wl[1])], axis=1)    # [128, 256]
    wo = to16(np.asarray(Wo, np.float32).reshape(128, 1))      # [128, 1]
    # b1 is applied plain at the psz stage of layer 0 (after W1); b2/b3 ride
    # the gained aggregation epilogue
    b123 = np.stack([np.asarray(b, np.float32) * g
                     for b, g in zip((b1, b2, b3),
                                     (1.0, GAINS[1], GAINS[2]))])
    b123 = b123.astype(BF16_NP).reshape(1, 3 * HIDDEN)         # [1, 768]
    bl_a = np.asarray(bl, np.float32).reshape(128, 1)          # [128, 1]
    bo_a = np.asarray(bo, np.float32).reshape(1, 1)
    return dict(w1=w1, w2=w2, w3=w3, wl=wl, wo=wo, b123=b123,
                bl=bl_a, bo=bo_a)


# -------------------------------------------------------------- bass kernel --

def build(cfg):
    """Build the SPMD Bass program (same graph on all 8 cores)."""
    T_half = cfg["T_half"]
    gw = cfg["gw"]
    calls = cfg["calls"]
    seg = cfg["seg"]
    T_tiles = 2 * T_half
    n_c = T_tiles * 128
    nh = T_half * 128
    t_kt = int(sum(a + b for a, b in gw))
    n_groups = (T_tiles + GS - 1) // GS

    nc = bacc.Bacc(None, target_bir_lowering=False,
                   dynamic_dma_scratch_size=49152,
                   num_swdge_queues=4)
    P = {}
    P["xloc"] = nc.declare_dram_parameter("xloc", [128, T_tiles * N_FEAT],
                                          FP8, False)
    xtab = [nc.declare_dram_parameter(f"xtab{w}", [N_CORES * nh, HIDDEN],
                                      FP8, False) for w in range(2)]
    P["disg"] = nc.declare_dram_parameter("disg", [128, 3 * T_tiles], F32, False)
    P["disu"] = nc.declare_dram_parameter("disu", [128, 3 * T_tiles], F32, False)
    P["invdis"] = nc.declare_dram_parameter("invdis", [1, n_c], BF16, False)
    P["batchloc"] = nc.declare_dram_parameter("batchloc", [128, T_tiles], F32, False)
    P["gidx"] = nc.declare_dram_parameter("gidx", [128, t_kt * 8], I16, False)
    selh_d = nc.declare_dram_parameter("selh", [128, t_kt * 128], FP8, False)
    P["rec"] = nc.declare_dram_parameter("rec", [128, GPC], F32, False)
    P["w1"] = nc.declare_dram_parameter("w1", [128, HIDDEN], BF16, False)
    P["w2"] = nc.declare_dram_parameter("w2", [128, 2 * HIDDEN], BF16, False)
    P["w3"] = nc.declare_dram_parameter("w3", [128, 2 * HIDDEN], BF16, False)
    P["wl"] = nc.declare_dram_parameter("wl", [128, 256], BF16, False)
    P["wo"] = nc.declare_dram_parameter("wo", [128, 1], BF16, False)
    P["b123"] = nc.declare_dram_parameter("b123", [1, 3 * HIDDEN], BF16, False)
    P["bl"] = nc.declare_dram_parameter("bl", [128, 1], F32, False)
    P["bo"] = nc.declare_dram_parameter("bo", [1, 1], F32, False)
    out_ext = nc.declare_dram_parameter("out", [1, GPC], F32, True)

    rg = [list(range(N_CORES))]
    AF = mybir.ActivationFunctionType
    OP = mybir.AluOpType

    with tile.TileContext(nc) as tc:
        with (
            tc.tile_pool(name="const", bufs=1) as cpool,
            tc.tile_pool(name="big", bufs=1) as bigpool,
            tc.tile_pool(name="work", bufs=3) as wpool,
            tc.tile_pool(name="gath", bufs=3) as gpool,
            tc.tile_pool(name="sel", bufs=2) as spool,
            tc.tile_pool(name="ps", bufs=2, space="PSUM") as pspool,
            tc.tile_pool(name="psa", bufs=3, space="PSUM") as papool,
            tc.tile_pool(name="pspool", bufs=1, space="PSUM") as ppool,
            tc.tile_pool(name="dram", bufs=2, space="DRAM") as dpool,
        ):
            # ---- constants / parameter loads (once) ----
            iota_t = cpool.tile([128, 128], F32)
            nc.gpsimd.iota(iota_t[:], pattern=[[1, 128]], base=0,
                           channel_multiplier=0,
                           allow_small_or_imprecise_dtypes=True)
            ident = cpool.tile([128, 128], BF16)
            make_identity(nc, ident[:])
            ident8 = cpool.tile([128, 128], GDT)
            nc.vector.tensor_copy(ident8[:], ident[:])
            ones1 = cpool.tile([1, 128], BF16)
            nc.vector.memset(ones1[:], 1.0)

            def load(name, shape, dt):
                t = cpool.tile(list(shape), dt, name=f"sb_{name}")
                nc.sync.dma_start(out=t[:], in_=P[name][:, :])
                return t

            xloc_sb = bigpool.tile([128, T_tiles * N_FEAT], GDT)
            nc.sync.dma_start(out=xloc_sb[:], in_=P["xloc"][:, :])
            disg_sb = load("disg", (128, 3 * T_tiles), F32)
            disu_sb = load("disu", (128, 3 * T_tiles), F32)
            invdis_sb = load("invdis", (1, n_c), BF16)
            batchloc_sb = load("batchloc", (128, T_tiles), F32)
            gidx_sb = bigpool.tile([128, t_kt * 8], I16)
            nc.sync.dma_start(out=gidx_sb[:], in_=P["gidx"][:, :])
            rec_sb = load("rec", (128, GPC), F32)
            w1_sb = load("w1", (128, HIDDEN), BF16)
            w2_sb = load("w2", (128, 2 * HIDDEN), BF16)
            w3_sb = load("w3", (128, 2 * HIDDEN), BF16)
            wl_sb = load("wl", (128, 256), BF16)
            wo_sb = load("wo", (128, 1), BF16)
            b123_sb = load("b123", (1, 3 * HIDDEN), BF16)
            bl_sb = load("bl", (128, 1), F32)
            bo_sb = load("bo", (1, 1), F32)

            # persistent transposed activations for the dense matmuls, and
            # the SBUF-resident z store (self-loop source)
            hT0 = bigpool.tile([128, n_c], BF16)
            hT1 = bigpool.tile([128, n_c], BF16)
            zstore = bigpool.tile([128, T_tiles * HIDDEN], GDT)

            pool_ps = [None, None]      # [chunk] psum tiles for poolT, per win
            out_sb = cpool.tile([1, GPC], F32)
            qno = [0]

            def emit_dense(ln, t, zloc_n):
                """z_{ln} for tile t -> zstore + DRAM table (pre-AllGather)."""
                sl = slice(t * 128, (t + 1) * 128)
                w_sb = (w1_sb, w2_sb, w3_sb)[ln]
                psz = pspool.tile([128, HIDDEN], F32, tag="mm", name="psz")
                nc.tensor.matmul(psz[:], lhsT=hT0[:, sl],
                                 rhs=w_sb[:, 0:HIDDEN],
                                 start=True, stop=False)
                nc.tensor.matmul(psz[:], lhsT=hT1[:, sl],
                                 rhs=w_sb[:, HIDDEN:2 * HIDDEN],
                                 start=False, stop=True)
                zsl = zstore[:, t * HIDDEN:(t + 1) * HIDDEN]
                nc.scalar.activation(
                    zsl, psz[:], AF.Copy,
                    scale=disg_sb[:, ln * T_tiles + t:ln * T_tiles + t + 1])
                hsl = slice((t % T_half) * 128, (t % T_half + 1) * 128)
                nc.sync.dma_start(out=zloc_n[t // T_half][hsl, :], in_=zsl)

            def alloc_zloc():
                return [dpool.tile([nh, HIDDEN], GDT, tag=f"zloc{w}",
                                   name=f"zloc{w}") for w in range(2)]

            def emit_ag(w, zloc_n, zfull_n):
                zfull_n[w] = dpool.tile([N_CORES * nh, HIDDEN], GDT,
                                        tag=f"zfull{w}", name=f"zfull{w}",
                                        addr_space="Shared")
                nc.gpsimd.collective_compute(
                    "AllGather", OP.bypass, replica_groups=rg,
                    ins=[zloc_n[w].opt()], outs=[zfull_n[w].opt()])

            def emit_gather(g, w, zfull_c):
                c0, nk = calls[2 * g + w]
                if nk == 0:
                    return None
                gt = gpool.tile([128, nk * HIDDEN], GDT, tag=f"gath{w}",
                                name=f"gath{w}", bufs=(4 if w == 0 else 3))
                gv = gt[:].rearrange("p (k h) -> p k h", h=HIDDEN)
                nidx = nk * 128
                nc.gpsimd.dma_gather(
                    out_ap=gv, in_ap=zfull_c[w][:, :],
                    idxs_ap=gidx_sb[:, c0 * 8:(c0 + nk) * 8],
                    num_idxs=nidx, num_idxs_reg=nidx,
                    elem_size=HIDDEN, single_packet=False,
                    queue_num=qno[0] % 4)
                qno[0] += 1
                return (gt, c0)

            def emit_sel(g):
                c0 = calls[2 * g][0]
                nksum = calls[2 * g][1] + calls[2 * g + 1][1]
                sel_sb = spool.tile([128, nksum * 128], GDT, tag="sel",
                                    name="sel_sb", bufs=3)
                nc.sync.dma_start(
                    out=sel_sb[:],
                    in_=selh_d[:, c0 * 128:(c0 + nksum) * 128])
                return sel_sb

            # layer 0 gathers straight from the host-replicated x tables —
            # no dense prologue and no layer-0 AllGather
            zfull_cur = xtab
            pend_ag = None          # deferred B-window AllGather of this layer
            ag_a_group = (T_half - 1) // GS + 2

            for layer in range(3):
                # software-pipelined gathers: A calls for the first three
                # groups go first, and only then the B-table AllGather
                # trigger (whose input-store wait would otherwise block the
                # gpsimd stream) followed by the B calls that consume it
                pend_g = {}
                pend_s = {}
                for ga in range(min(3, n_groups)):
                    pend_g[(ga, 0)] = emit_gather(ga, 0, zfull_cur)
                if pend_ag is not None:
                    emit_ag(1, *pend_ag)
                    pend_ag = None
                pend_g[(0, 1)] = emit_gather(0, 1, zfull_cur)
                pend_s[0] = emit_sel(0)
                if n_groups > 1:
                    pend_g[(1, 1)] = emit_gather(1, 1, zfull_cur)
                    pend_s[1] = emit_sel(1)

                ag_done = [False, False]
                zloc_nn = None
                if layer < 2:
                    zloc_nn = alloc_zloc()
                zfull_next = [None, None]

                b_row = b123_sb[0:1, layer * HIDDEN:(layer + 1) * HIDDEN]
                for g in range(n_groups):
                    ts = range(g * GS, min((g + 1) * GS, T_tiles))
                    if g + 3 < n_groups:
                        pend_g[(g + 3, 0)] = emit_gather(g + 3, 0, zfull_cur)
                    if g + 2 < n_groups:
                        pend_g[(g + 2, 1)] = emit_gather(g + 2, 1, zfull_cur)
                        pend_s[g + 2] = emit_sel(g + 2)
                    gath = [pend_g.pop((g, 0)), pend_g.pop((g, 1))]
                    sel_sb = pend_s.pop(g)
                    c0 = calls[2 * g][0]

                    for t in ts:
                        sl = slice(t * 128, (t + 1) * 128)
                        agw = HIDDEN if layer else N_FEAT
                        psa = papool.tile([128, agw], F32, tag="psa")
                        if layer:
                            # bias as (1/dis)[dst] (x) b -> exact norm
                            nc.tensor.matmul(
                                psa[:], lhsT=invdis_sb[0:1, sl], rhs=b_row,
                                start=True, stop=False)
                        # self-loop term: agg[v] += z[v] (x[v] for layer 0)
                        nc.tensor.matmul(
                            psa[:], lhsT=ident8[:],
                            rhs=(zstore[:, t * HIDDEN:(t + 1) * HIDDEN]
                                 if layer else
                                 xloc_sb[:, t * N_FEAT:(t + 1) * N_FEAT]),
                            start=(layer == 0),
                            stop=(gw[t][0] + gw[t][1] == 0))
                        for w in range(2):
                            gwt = gw[t][w]
                            if gwt == 0:
                                continue
                            gt, gc0 = gath[w]
                            off = seg[t][w] - gc0
                            last = (w == 1) or gw[t][1] == 0
                            for j in range(gwt):
                                nc.tensor.matmul(
                                    psa[:],
                                    lhsT=sel_sb[:, (seg[t][w] - c0 + j) * 128:
                                                (seg[t][w] - c0 + j + 1) * 128],
                                    rhs=gt[:, (off + j) * HIDDEN:
                                           (off + j) * HIDDEN + agw],
                                    start=False,
                                    stop=(last and j == gwt - 1))
                        if layer == 0:
                            # u = dis * agg; h1 = relu(uT^T @ W1 + b1)
                            u_sb = wpool.tile([128, N_FEAT], BF16, tag="u")
                            nc.scalar.activation(
                                u_sb[:], psa[:], AF.Copy,
                                scale=disu_sb[:, t:t + 1])
                            psu0 = pspool.tile([N_FEAT, 128], BF16, tag="mm",
                                               name="psu0")
                            nc.tensor.transpose(psu0[:], u_sb[:], ident[:])
                            uT_sb = wpool.tile([N_FEAT, 128], BF16, tag="uT")
                            nc.vector.tensor_copy(uT_sb[:], psu0[:])
                            psz1 = papool.tile([128, HIDDEN], F32, tag="psa",
                                               name="psz1")
                            nc.tensor.matmul(psz1[:], lhsT=uT_sb[:],
                                             rhs=w1_sb[:N_FEAT, :],
                                             start=True, stop=False)
                            nc.tensor.matmul(psz1[:], lhsT=ones1[:],
                                             rhs=b_row, start=False, stop=True)
                            psa = psz1
                        # h = relu(dis * agg + b)   (layer 0: relu(z1 + b1))
                        h_sb = wpool.tile([128, HIDDEN], BF16, tag="h")
                        if layer:
                            nc.scalar.activation(
                                h_sb[:], psa[:], AF.Relu,
                                scale=disu_sb[:, layer * T_tiles + t:
                                              layer * T_tiles + t + 1])
                        else:
                            nc.scalar.activation(h_sb[:], psa[:], AF.Relu)
                        if layer < 2:
                            for cch in range(2):
                                pst = pspool.tile([128, 128], BF16, tag="mm")
                                nc.tensor.transpose(
                                    pst[:], h_sb[:, cch * 128:(cch + 1) * 128],
                                    ident[:])
                                hT = (hT0, hT1)[cch]
                                nc.vector.tensor_copy(hT[:, sl], pst[:])
                            # next layer's z for this tile rides along, so
                            # the AllGathers can overlap this layer's tail
                            emit_dense(layer + 1, t, zloc_nn)
                        else:
                            win = t // T_half
                            first = (t % T_half) == 0
                            last = (t % T_half) == T_half - 1
                            if first:
                                pool_ps[0] = ppool.tile([128, 128], F32,
                                                        name="poolT0",
                                                        tag="poolT0", bufs=1)
                                pool_ps[1] = ppool.tile([128, 128], F32,
                                                        name="poolT1",
                                                        tag="poolT1", bufs=1)
                            selp = spool.tile([128, 128], BF16, tag="selp")
                            nc.vector.tensor_tensor(
                                selp[:],
                                batchloc_sb[:, t:t + 1].to_broadcast([128, 128]),
                                iota_t[:], op=OP.is_equal)
                            for cch in range(2):
                                nc.tensor.matmul(
                                    pool_ps[cch][:],
                                    lhsT=h_sb[:, cch * 128:(cch + 1) * 128],
                                    rhs=selp[:], start=first, stop=last)
                            if last:
                                # ---- head for this window of 128 graphs ----
                                rrow = rec_sb[:, win * GPW:(win + 1) * GPW]
                                psu = papool.tile([128, GPW], F32, tag="head",
                                                  bufs=1)
                                for cch in range(2):
                                    gT = wpool.tile([128, GPW], BF16,
                                                    tag="gT")
                                    nc.vector.tensor_tensor(
                                        gT[:], pool_ps[cch][:, :GPW],
                                        rrow, op=OP.mult)
                                    nc.tensor.matmul(
                                        psu[:],
                                        lhsT=wl_sb[:, cch * 128:(cch + 1) * 128],
                                        rhs=gT[:], start=(cch == 0),
                                        stop=(cch == 1))
                                uT = wpool.tile([128, GPW], BF16, tag="uT")
                                nc.scalar.activation(uT[:], psu[:], AF.Relu,
                                                     bias=bl_sb[:, 0:1])
                                pso = papool.tile([1, GPW], F32, tag="head",
                                                  bufs=1)
                                nc.tensor.matmul(pso[:], lhsT=wo_sb[:, 0:1],
                                                 rhs=uT[:], start=True,
                                                 stop=True)
                                nc.vector.tensor_scalar(
                                    out_sb[0:1, win * GPW:(win + 1) * GPW],
                                    pso[:], bo_sb[0:1, 0:1], None, op0=OP.add)
                    if layer < 2:
                        if not ag_done[0] and g >= ag_a_group:
                            emit_ag(0, zloc_nn, zfull_next)
                            ag_done[0] = True
                        if g == n_groups - 1:
                            pend_ag = (zloc_nn, zfull_next)
                zfull_cur = zfull_next
            nc.sync.dma_start(out=out_ext[:, :], in_=out_sb[:])
    nc.finalize()
    return nc


# ------------------------------------------------------------------ runner --

_CACHE = {}


def _get_program(cfg):
    key = (cfg["T_half"], cfg["gw"])
    if key not in _CACHE:
        _CACHE[key] = build(cfg)
    return _CACHE[key]


def kernel(x, edge_index, batch, W1, b1, W2, b2, W3, b3, Wl, bl, Wo, bo):
    from concourse.bass_utils import run_bass_kernel_spmd

    cfg, arrays = preprocess(x, edge_index, batch)
    wts = pack_weights(W1, b1, W2, b2, W3, b3, Wl, bl, Wo, bo)
    nc = _get_program(cfg)

    in_maps = []
    for c in range(N_CORES):
        m = {k: np.ascontiguousarray(v[c]) for k, v in arrays.items()}
        m.update(wts)
        in_maps.append(m)

    res = run_bass_kernel_spmd(nc, in_maps, core_ids=list(range(N_CORES)))
    outs = res.results
    out = np.concatenate([outs[c]["out"].reshape(GPC) for c in range(N_CORES)])
    return out.reshape(N_GRAPHS, 1).astype(np.float32)
